# revision 1
# baseline (speedup 1.0000x reference)
"""MultiHeadAttention (cosine-sim, no softmax) + residual + LayerNorm on 8 TRN2 cores.

Reference math (per sample row x of q/k/v, D=2048, H=16, HD=128):
  qp = q @ Wq + bq   (kept as residual)
  kp = k @ Wk + bk ; vp = v @ Wv + bv
  per head h: qn = qh/||qh||, kn = kh/||kh||
  s[h,g] = (qn_h . kn_g) / HD          # [16,16] per sample
  o[h] = sum_g s[h,g] * vh_g           # [16,128]
  o_flat[hd*16+h] = o[h,hd]            # interleaved flatten
  o2 = o_flat @ Wo + bo
  x = qp + o2 ; out = layernorm(x) * gamma + beta

Sharding: pure data-parallel over batch (4096 rows/core), weights replicated.

Device strategy (per core), all heavy matmuls in bf16 with fp32 PSUM accum:
  - Host pre-transposes q,k,v -> [D, B] bf16 so activation blocks load as
    [k,b] tiles directly (PE contracts over the partition dim).
  - qp, kp computed in natural space: psum[128b, 512n] += qT_blk.T @ W[k,n512].
  - vp computed transposed: psum[128n, 512b] += Wv[k,n128].T @ vT[k,b512],
    giving vhT[hd, g, b] directly (head g = row-block g of vpT).
  - Per-head normalize of qp/kp in natural space (free-dim reduce), then PE
    transposes to qnT/knT[hd, h, b].
  - Scores for 8 samples at once: ST[(g,s),(h,s')] = knT_s8.T @ qnT_s8; the
    block-diagonal (s==s') entries are the real scores; multiply by a host
    constant mask (1/HD on diag blocks, 0 elsewhere) -> masked bf16 ST.
  - o for 8 samples in one matmul: oT[hd,(h,s)] = V_blk.T @ ST_masked where
    V_blk[(g,s),hd] is a PE transpose of a vhT slice. Cross-sample terms
    vanish because ST_masked is block-diagonal.
  - Output projection uses host-permuted Wo' (row hd*16+h -> h*128+hd) so
    o2[128b,512n] += oT[:,h,bt].T @ Wo'[h*128:,n512] accumulated over h.
  - Residual add + layernorm fused on-chip; fp32 output.
"""

from contextlib import ExitStack

import numpy as np
import ml_dtypes

import concourse.bass as bass
import concourse.bacc as bacc
import concourse.mybir as mybir
import concourse.tile as tile
from concourse.bass_utils import run_bass_kernel_spmd

BF16 = mybir.dt.bfloat16
F32 = mybir.dt.float32

B, D, H, HD = 32768, 2048, 16, 128
NCORES = 8
EPS = 1e-5
CHUNK = 512          # samples per chunk (8 chunks per core at BL=4096)
NG = D // 512        # 4 n-chunks of 512 columns
KO = D // 128        # 16 contraction blocks
SBLK = 8             # samples per attention block matmul (8*16 = 128)


def build_bass(bl, repeat=1):
    """Build the per-core Bass program for bl batch rows (bl % CHUNK == 0)."""
    nc = bacc.Bacc()

    qT = nc.dram_tensor("qT", [D, bl], BF16, kind="ExternalInput")
    kT = nc.dram_tensor("kT", [D, bl], BF16, kind="ExternalInput")
    vT = nc.dram_tensor("vT", [D, bl], BF16, kind="ExternalInput")
    Wq = nc.dram_tensor("Wq", [D, D], BF16, kind="ExternalInput")
    Wk = nc.dram_tensor("Wk", [D, D], BF16, kind="ExternalInput")
    Wv = nc.dram_tensor("Wv", [D, D], BF16, kind="ExternalInput")
    Wo = nc.dram_tensor("Wo", [D, D], BF16, kind="ExternalInput")  # permuted on host
    bq = nc.dram_tensor("bq", [1, D], BF16, kind="ExternalInput")
    bk = nc.dram_tensor("bk", [1, D], BF16, kind="ExternalInput")
    bo = nc.dram_tensor("bo", [1, D], BF16, kind="ExternalInput")
    bv = nc.dram_tensor("bv", [128, KO], F32, kind="ExternalInput")  # [p, nb] layout
    gamma = nc.dram_tensor("gamma", [1, D], F32, kind="ExternalInput")
    beta = nc.dram_tensor("beta", [1, D], F32, kind="ExternalInput")
    ident = nc.dram_tensor("ident", [128, 128], BF16, kind="ExternalInput")
    mask = nc.dram_tensor("mask", [128, 128], BF16, kind="ExternalInput")
    ones = nc.dram_tensor("ones", [1, 128], BF16, kind="ExternalInput")
    out = nc.dram_tensor("out", [bl, D], F32, kind="ExternalOutput")

    nchunks = bl // CHUNK
    NBT = CHUNK // 128  # b-tiles per chunk

    with tile.TileContext(nc) as tc, ExitStack() as ctx:
        consts = ctx.enter_context(tc.tile_pool(name="consts", bufs=1))
        qkvT_pool = ctx.enter_context(tc.tile_pool(name="qkvT", bufs=1))
        wko_pool = ctx.enter_context(tc.tile_pool(name="wko", bufs=8))
        chunk_pool = ctx.enter_context(tc.tile_pool(name="chunkbuf", bufs=1))
        trans_pool = ctx.enter_context(tc.tile_pool(name="trans", bufs=3))
        small_pool = ctx.enter_context(tc.tile_pool(name="small", bufs=4))
        out_pool = ctx.enter_context(tc.tile_pool(name="outb", bufs=3))
        proj_psum = ctx.enter_context(tc.tile_pool(name="proj_psum", bufs=4, space="PSUM"))
        att_psum = ctx.enter_context(tc.tile_pool(name="att_psum", bufs=4, space="PSUM"))

        # ---- constants ----
        ident_sb = consts.tile([128, 128], BF16)
        nc.sync.dma_start(out=ident_sb, in_=ident[:, :])
        mask_sb = consts.tile([128, 128], BF16)
        nc.sync.dma_start(out=mask_sb, in_=mask[:, :])
        ones_sb = consts.tile([1, 128], BF16)
        nc.sync.dma_start(out=ones_sb, in_=ones[:, :])
        bq_sb = consts.tile([1, D], BF16)
        nc.sync.dma_start(out=bq_sb, in_=bq[:, :])
        bk_sb = consts.tile([1, D], BF16)
        nc.sync.dma_start(out=bk_sb, in_=bk[:, :])
        bo_sb = consts.tile([1, D], BF16)
        nc.sync.dma_start(out=bo_sb, in_=bo[:, :])
        bv_sb = consts.tile([128, KO], F32)
        nc.sync.dma_start(out=bv_sb, in_=bv[:, :])
        eps_sb = consts.tile([128, 1], F32)
        nc.vector.memset(eps_sb, EPS)
        # gamma/beta broadcast across all 128 partitions (DMA partition step 0)
        g_ap = gamma[:, :]
        gamma_sb = consts.tile([128, D], F32)
        nc.sync.dma_start(
            out=gamma_sb,
            in_=bass.AP(tensor=g_ap.tensor, offset=g_ap.offset,
                        ap=[[0, 128], [1, D]]),
        )
        b_ap = beta[:, :]
        beta_sb = consts.tile([128, D], F32)
        nc.sync.dma_start(
            out=beta_sb,
            in_=bass.AP(tensor=b_ap.tensor, offset=b_ap.offset,
                        ap=[[0, 128], [1, D]]),
        )

        qT3 = qT.rearrange("(ko p) b -> p ko b", p=128)
        kT3 = kT.rearrange("(ko p) b -> p ko b", p=128)
        vT3 = vT.rearrange("(ko p) b -> p ko b", p=128)
        Wq3 = Wq.rearrange("(ko p) n -> p ko n", p=128)
        Wk3 = Wk.rearrange("(ko p) n -> p ko n", p=128)
        Wv3 = Wv.rearrange("(ko p) n -> p ko n", p=128)
        Wo3 = Wo.rearrange("(ko p) n -> p ko n", p=128)

        for _rep in range(repeat):
          for c in range(nchunks):
            b0 = c * CHUNK
            # chunk-resident activation inputs [128, KO, CHUNK] bf16
            qT_sb = qkvT_pool.tile([128, KO, CHUNK], BF16, tag="qT")
            nc.sync.dma_start(out=qT_sb, in_=qT3[:, :, b0:b0 + CHUNK])
            kT_sb = qkvT_pool.tile([128, KO, CHUNK], BF16, tag="kT")
            nc.sync.dma_start(out=kT_sb, in_=kT3[:, :, b0:b0 + CHUNK])
            vT_sb = qkvT_pool.tile([128, KO, CHUNK], BF16, tag="vT")
            nc.sync.dma_start(out=vT_sb, in_=vT3[:, :, b0:b0 + CHUNK])

            # chunk-lifetime buffers
            # qnT/knT/vhT use interleaved [hd, blk, h, s] layout (b = blk*8+s)
            # so a per-block slice [:, blk] is one contiguous 128-wide free dim
            # as required by matmul operands.
            NBLK = CHUNK // SBLK
            qp_sb = chunk_pool.tile([128, NBT, D], F32, tag="qp")      # residual (becomes x)
            qnT_sb = chunk_pool.tile([128, NBLK, H, SBLK], BF16, tag="qnT")
            knT_sb = chunk_pool.tile([128, NBLK, H, SBLK], BF16, tag="knT")
            vhT_sb = chunk_pool.tile([128, NBLK, H, SBLK], BF16, tag="vhT")
            oT_sb = chunk_pool.tile([128, H, CHUNK], BF16, tag="oT")

            # ---- q,k projections (natural space) + normalize + transpose ----
            for ng in range(NG):
                n0 = ng * 512
                for (xT_sb, W3, b_sb, is_q) in (
                    (qT_sb, Wq3, bq_sb, True),
                    (kT_sb, Wk3, bk_sb, False),
                ):
                    ps_list = [proj_psum.tile([128, 512], F32, tag="pp",
                                               name=f"pp_{c}_{ng}_{is_q}_{bt}")
                               for bt in range(NBT)]
                    for ko in range(KO):
                        w_sb = wko_pool.tile([128, 512], BF16, tag="w")
                        nc.scalar.dma_start(out=w_sb, in_=W3[:, ko, n0:n0 + 512])
                        for bt in range(NBT):
                            nc.tensor.matmul(
                                ps_list[bt],
                                xT_sb[:, ko, bt * 128:(bt + 1) * 128],
                                w_sb, start=(ko == 0), stop=False)
                    for bt in range(NBT):
                        ps = ps_list[bt]
                        # bias via K=1 ones-row matmul (broadcast along partitions)
                        nc.tensor.matmul(ps, ones_sb, b_sb[:, n0:n0 + 512],
                                         start=False, stop=True)
                        if is_q:
                            # keep fp32 residual
                            nc.scalar.copy(out=qp_sb[:, bt, n0:n0 + 512], in_=ps)
                        # per-head 1/||.|| for the 4 heads in this n-chunk
                        # (single-input ACT Square + accumulate; DVE can't
                        # read the same PSUM tile twice)
                        rr = small_pool.tile([128, 4], F32, tag="rr")
                        scratch = trans_pool.tile([128, 128], BF16, tag="scr")
                        for h4 in range(4):
                            nc.scalar.activation(
                                out=scratch,
                                in_=ps[:, h4 * 128:(h4 + 1) * 128],
                                func=mybir.ActivationFunctionType.Square,
                                accum_out=rr[:, h4:h4 + 1])
                        nc.scalar.activation(out=rr, in_=rr,
                                             func=mybir.ActivationFunctionType.Sqrt)
                        nc.vector.reciprocal(out=rr, in_=rr)
                        # normalized bf16 copy (natural layout)
                        nrm = trans_pool.tile([128, 512], BF16, tag="nrm")
                        for h4 in range(4):
                            nc.vector.tensor_scalar_mul(
                                out=nrm[:, h4 * 128:(h4 + 1) * 128],
                                in0=ps[:, h4 * 128:(h4 + 1) * 128],
                                scalar1=rr[:, h4:h4 + 1])
                        # transpose each head block -> [hd, b], scatter into
                        # interleaved [hd, blk, h, s] layout
                        dstT = qnT_sb if is_q else knT_sb
                        for h4 in range(4):
                            tp = att_psum.tile([128, 128], BF16, tag="ap")
                            nc.tensor.transpose(
                                tp, nrm[:, h4 * 128:(h4 + 1) * 128], ident_sb)
                            nc.scalar.copy(
                                out=dstT[:, bt * 16:(bt + 1) * 16, ng * 4 + h4, :],
                                in_=tp[:, :].rearrange(
                                    "p (blk s) -> p blk s", s=SBLK))

            # ---- v projection (transposed space) -> vhT ----
            for g in range(H):
                ps_v = [proj_psum.tile([128, 256], F32, tag="pp",
                                        name=f"pv_{c}_{g}_{half}")
                        for half in range(CHUNK // 256)]
                for ko in range(KO):
                    w_sb = wko_pool.tile([128, 128], BF16, tag="wv")
                    nc.scalar.dma_start(out=w_sb,
                                        in_=Wv3[:, ko, g * 128:(g + 1) * 128])
                    for half in range(CHUNK // 256):
                        nc.tensor.matmul(
                            ps_v[half], w_sb,
                            vT_sb[:, ko, half * 256:(half + 1) * 256],
                            start=(ko == 0), stop=(ko == KO - 1))
                for half in range(CHUNK // 256):
                    # add per-partition bias while copying psum->sbuf bf16
                    nc.scalar.activation(
                        out=vhT_sb[:, half * 32:(half + 1) * 32, g, :],
                        in_=ps_v[half][:, :].rearrange(
                            "p (blk s) -> p blk s", s=SBLK),
                        func=mybir.ActivationFunctionType.Identity,
                        bias=bv_sb[:, g:g + 1], scale=1.0)

            # ---- attention: scores + o, 8 samples per matmul ----
            for blk in range(CHUNK // SBLK):
                s0 = blk * SBLK
                # ST[(g,s),(h,s')] = knT_s8.T @ qnT_s8   (contraction over hd)
                st_ps = att_psum.tile([128, 128], F32, tag="ap")
                nc.tensor.matmul(
                    st_ps,
                    knT_sb[:, blk].rearrange("p h s -> p (h s)"),
                    qnT_sb[:, blk].rearrange("p h s -> p (h s)"),
                    start=True, stop=True)
                st_sb = trans_pool.tile([128, 128], BF16, tag="st")
                nc.vector.tensor_mul(out=st_sb, in0=st_ps, in1=mask_sb)
                # V_blk[(g,s),hd] = transpose(vhT[:, blk])
                vb_ps = att_psum.tile([128, 128], BF16, tag="ap")
                nc.tensor.transpose(
                    vb_ps, vhT_sb[:, blk].rearrange("p h s -> p (h s)"), ident_sb)
                vb_sb = trans_pool.tile([128, 128], BF16, tag="vb")
                nc.scalar.copy(out=vb_sb, in_=vb_ps)
                # oT[hd,(h,s)] = V_blk.T @ ST_masked
                o_ps = att_psum.tile([128, 128], F32, tag="ap")
                nc.tensor.matmul(o_ps, vb_sb, st_sb, start=True, stop=True)
                nc.scalar.copy(
                    out=oT_sb[:, :, s0:s0 + SBLK],
                    in_=o_ps[:, :].rearrange("p (h s) -> p h s", h=H))

            # ---- output projection + residual + layernorm ----
            for ng in range(NG):
                n0 = ng * 512
                ps_list = [proj_psum.tile([128, 512], F32, tag="pp",
                                           name=f"po_{c}_{ng}_{bt}")
                           for bt in range(NBT)]
                for h in range(H):
                    w_sb = wko_pool.tile([128, 512], BF16, tag="w")
                    nc.scalar.dma_start(out=w_sb, in_=Wo3[:, h, n0:n0 + 512])
                    for bt in range(NBT):
                        nc.tensor.matmul(
                            ps_list[bt], oT_sb[:, h, bt * 128:(bt + 1) * 128],
                            w_sb, start=(h == 0), stop=False)
                for bt in range(NBT):
                    ps = ps_list[bt]
                    nc.tensor.matmul(ps, ones_sb, bo_sb[:, n0:n0 + 512],
                                     start=False, stop=True)
                    # x = qp + o2 (in place into qp_sb)
                    nc.vector.tensor_add(
                        out=qp_sb[:, bt, n0:n0 + 512],
                        in0=qp_sb[:, bt, n0:n0 + 512], in1=ps)

            for bt in range(NBT):
                x_ap = qp_sb[:, bt, :]
                stats = small_pool.tile([128, 4, 6], F32, tag="bn")
                for sg in range(4):
                    nc.vector.bn_stats(out=stats[:, sg, :],
                                       in_=x_ap[:, sg * 512:(sg + 1) * 512])
                mv = small_pool.tile([128, 2], F32, tag="mv")
                nc.vector.bn_aggr(out=mv, in_=stats)
                rstd = small_pool.tile([128, 1], F32, tag="rstd")
                nc.scalar.activation(out=rstd, in_=mv[:, 1:2],
                                     func=mybir.ActivationFunctionType.Sqrt,
                                     bias=eps_sb, scale=1.0)
                nc.vector.reciprocal(out=rstd, in_=rstd)
                for ng in range(NG):
                    n0 = ng * 512
                    ot = out_pool.tile([128, 512], F32, tag="ot")
                    # (x - mu) * rstd
                    nc.vector.tensor_scalar(
                        out=ot, in0=x_ap[:, n0:n0 + 512],
                        scalar1=mv[:, 0:1], scalar2=rstd,
                        op0=mybir.AluOpType.subtract,
                        op1=mybir.AluOpType.mult)
                    # * gamma
                    nc.vector.tensor_mul(out=ot, in0=ot,
                                         in1=gamma_sb[:, n0:n0 + 512])
                    # + beta
                    nc.gpsimd.tensor_add(out=ot, in0=ot,
                                         in1=beta_sb[:, n0:n0 + 512])
                    nc.sync.dma_start(
                        out=out[b0 + bt * 128:b0 + (bt + 1) * 128, n0:n0 + 512],
                        in_=ot)

    nc.compile()
    return nc


def _prep_host_inputs(q, k, v, Wq, bq, Wk, bk, Wv, bv, Wo, bo, gamma, beta):
    bf = ml_dtypes.bfloat16
    qT = np.ascontiguousarray(q.T).astype(bf)
    kT = np.ascontiguousarray(k.T).astype(bf)
    vT = np.ascontiguousarray(v.T).astype(bf)
    # Wo' row h*128+hd  <- Wo row hd*16+h
    hh, dd = np.divmod(np.arange(D), HD)     # d' = h*HD+hd -> h=hh, hd=dd
    src = dd * H + hh
    Wo_p = np.ascontiguousarray(Wo[src, :]).astype(bf)
    # block-diag mask, 1/HD on (r,c) where r%8 == c%8
    r = np.arange(128)
    m = (r[:, None] % SBLK == r[None, :] % SBLK).astype(np.float32) / HD
    shared = {
        "Wq": np.ascontiguousarray(Wq).astype(bf),
        "Wk": np.ascontiguousarray(Wk).astype(bf),
        "Wv": np.ascontiguousarray(Wv).astype(bf),
        "Wo": Wo_p,
        "bq": bq.reshape(1, D).astype(bf),
        "bk": bk.reshape(1, D).astype(bf),
        "bo": bo.reshape(1, D).astype(bf),
        "bv": np.ascontiguousarray(bv.reshape(KO, 128).T).astype(np.float32),
        "gamma": gamma.reshape(1, D).astype(np.float32),
        "beta": beta.reshape(1, D).astype(np.float32),
        "ident": np.eye(128, dtype=bf),
        "mask": m.astype(bf),
        "ones": np.ones((1, 128), dtype=bf),
    }
    return qT, kT, vT, shared


def kernel(q, k, v, Wq, bq, Wk, bk, Wv, bv, Wo, bo, gamma, beta, _bl=None,
           _ncores=None, _trace=False):
    ncores = _ncores or NCORES
    bl = _bl or (q.shape[0] // ncores)
    qT, kT, vT, shared = _prep_host_inputs(
        q, k, v, Wq, bq, Wk, bk, Wv, bv, Wo, bo, gamma, beta)
    nc = build_bass(bl)
    in_maps = []
    for c in range(ncores):
        m = dict(shared)
        s = slice(c * bl, (c + 1) * bl)
        m["qT"] = np.ascontiguousarray(qT[:, s])
        m["kT"] = np.ascontiguousarray(kT[:, s])
        m["vT"] = np.ascontiguousarray(vT[:, s])
        in_maps.append(m)
    res = run_bass_kernel_spmd(nc, in_maps, core_ids=list(range(ncores)),
                               trace=_trace)
    outs = [r["out"] for r in res.results]
    full = np.concatenate(outs, axis=0)
    if _trace:
        kernel.last_results = res
    return full.astype(np.float32)



# revision 2
# speedup vs baseline: 16.6540x; 16.6540x over previous
"""MultiHeadAttention (cosine-sim, no softmax) + residual + LayerNorm on 8 TRN2 cores.

Reference math (per sample row x of q/k/v, D=2048, H=16, HD=128):
  qp = q @ Wq + bq   (kept as residual)
  kp = k @ Wk + bk ; vp = v @ Wv + bv
  per head h: qn = qh/||qh||, kn = kh/||kh||
  s[h,g] = (qn_h . kn_g) / HD          # [16,16] per sample
  o[h] = sum_g s[h,g] * vh_g           # [16,128]
  o_flat[hd*16+h] = o[h,hd]            # interleaved flatten
  o2 = o_flat @ Wo + bo
  x = qp + o2 ; out = layernorm(x) * gamma + beta

Sharding: pure data-parallel over batch (4096 rows/core), weights replicated.

Device strategy (per core), all heavy matmuls in bf16 with fp32 PSUM accum:
  - Host pre-transposes q,k,v -> [D, B] bf16 so activation blocks load as
    [k,b] tiles directly (PE contracts over the partition dim).
  - qp, kp computed in natural space: psum[128b, 512n] += qT_blk.T @ W[k,n512].
  - vp computed transposed: psum[128n, 512b] += Wv[k,n128].T @ vT[k,b512],
    giving vhT[hd, g, b] directly (head g = row-block g of vpT).
  - Per-head normalize of qp/kp in natural space (free-dim reduce), then PE
    transposes to qnT/knT[hd, h, b].
  - Scores for 8 samples at once: ST[(g,s),(h,s')] = knT_s8.T @ qnT_s8; the
    block-diagonal (s==s') entries are the real scores; multiply by a host
    constant mask (1/HD on diag blocks, 0 elsewhere) -> masked bf16 ST.
  - o for 8 samples in one matmul: oT[hd,(h,s)] = V_blk.T @ ST_masked where
    V_blk[(g,s),hd] is a PE transpose of a vhT slice. Cross-sample terms
    vanish because ST_masked is block-diagonal.
  - Output projection uses host-permuted Wo' (row hd*16+h -> h*128+hd) so
    o2[128b,512n] += oT[:,h,bt].T @ Wo'[h*128:,n512] accumulated over h.
  - Residual add + layernorm fused on-chip; fp32 output.
"""

from contextlib import ExitStack

import numpy as np
import ml_dtypes

import concourse.bass as bass
import concourse.bacc as bacc
import concourse.mybir as mybir
import concourse.tile as tile
from concourse.bass_utils import run_bass_kernel_spmd

BF16 = mybir.dt.bfloat16
F32 = mybir.dt.float32

B, D, H, HD = 32768, 2048, 16, 128
NCORES = 8
EPS = 1e-5
CHUNK = 512          # samples per chunk (8 chunks per core at BL=4096)
NG = D // 512        # 4 n-chunks of 512 columns
KO = D // 128        # 16 contraction blocks
SBLK = 8             # samples per attention block matmul (8*16 = 128)


def build_bass(bl, repeat=1):
    """Build the per-core Bass program for bl batch rows (bl % CHUNK == 0)."""
    nc = bacc.Bacc()

    qT = nc.dram_tensor("qT", [D, bl], BF16, kind="ExternalInput")
    kT = nc.dram_tensor("kT", [D, bl], BF16, kind="ExternalInput")
    vT = nc.dram_tensor("vT", [D, bl], BF16, kind="ExternalInput")
    Wq = nc.dram_tensor("Wq", [D, D], BF16, kind="ExternalInput")
    Wk = nc.dram_tensor("Wk", [D, D], BF16, kind="ExternalInput")
    Wv = nc.dram_tensor("Wv", [D, D], BF16, kind="ExternalInput")
    Wo = nc.dram_tensor("Wo", [D, D], BF16, kind="ExternalInput")  # permuted on host
    bq = nc.dram_tensor("bq", [1, D], BF16, kind="ExternalInput")
    bk = nc.dram_tensor("bk", [1, D], BF16, kind="ExternalInput")
    bo = nc.dram_tensor("bo", [1, D], BF16, kind="ExternalInput")
    bv = nc.dram_tensor("bv", [128, KO], F32, kind="ExternalInput")  # [p, nb] layout
    gamma = nc.dram_tensor("gamma", [1, D], F32, kind="ExternalInput")
    beta = nc.dram_tensor("beta", [1, D], F32, kind="ExternalInput")
    ident = nc.dram_tensor("ident", [128, 128], BF16, kind="ExternalInput")
    mask = nc.dram_tensor("mask", [128, 128], BF16, kind="ExternalInput")
    ones = nc.dram_tensor("ones", [1, 128], BF16, kind="ExternalInput")
    out = nc.dram_tensor("out", [bl, D], F32, kind="ExternalOutput")

    nchunks = bl // CHUNK
    NBT = CHUNK // 128  # b-tiles per chunk

    with tile.TileContext(nc) as tc, ExitStack() as ctx:
        consts = ctx.enter_context(tc.tile_pool(name="consts", bufs=1))
        qkvT_pool = ctx.enter_context(tc.tile_pool(name="qkvT", bufs=1))
        wko_pool = ctx.enter_context(tc.tile_pool(name="wko", bufs=8))
        chunk_pool = ctx.enter_context(tc.tile_pool(name="chunkbuf", bufs=1))
        trans_pool = ctx.enter_context(tc.tile_pool(name="trans", bufs=3))
        small_pool = ctx.enter_context(tc.tile_pool(name="small", bufs=4))
        out_pool = ctx.enter_context(tc.tile_pool(name="outb", bufs=3))
        proj_psum = ctx.enter_context(tc.tile_pool(name="proj_psum", bufs=4, space="PSUM"))
        att_psum = ctx.enter_context(tc.tile_pool(name="att_psum", bufs=4, space="PSUM"))

        # ---- constants ----
        ident_sb = consts.tile([128, 128], BF16)
        nc.sync.dma_start(out=ident_sb, in_=ident[:, :])
        mask_sb = consts.tile([128, 128], BF16)
        nc.sync.dma_start(out=mask_sb, in_=mask[:, :])
        ones_sb = consts.tile([1, 128], BF16)
        nc.sync.dma_start(out=ones_sb, in_=ones[:, :])
        bq_sb = consts.tile([1, D], BF16)
        nc.sync.dma_start(out=bq_sb, in_=bq[:, :])
        bk_sb = consts.tile([1, D], BF16)
        nc.sync.dma_start(out=bk_sb, in_=bk[:, :])
        bo_sb = consts.tile([1, D], BF16)
        nc.sync.dma_start(out=bo_sb, in_=bo[:, :])
        bv_sb = consts.tile([128, KO], F32)
        nc.sync.dma_start(out=bv_sb, in_=bv[:, :])
        eps_sb = consts.tile([128, 1], F32)
        nc.vector.memset(eps_sb, EPS)
        # gamma/beta broadcast across all 128 partitions (DMA partition step 0)
        g_ap = gamma[:, :]
        gamma_sb = consts.tile([128, D], F32)
        nc.sync.dma_start(
            out=gamma_sb,
            in_=bass.AP(tensor=g_ap.tensor, offset=g_ap.offset,
                        ap=[[0, 128], [1, D]]),
        )
        b_ap = beta[:, :]
        beta_sb = consts.tile([128, D], F32)
        nc.sync.dma_start(
            out=beta_sb,
            in_=bass.AP(tensor=b_ap.tensor, offset=b_ap.offset,
                        ap=[[0, 128], [1, D]]),
        )

        qT3 = qT.rearrange("(ko p) b -> p ko b", p=128)
        kT3 = kT.rearrange("(ko p) b -> p ko b", p=128)
        vT3 = vT.rearrange("(ko p) b -> p ko b", p=128)
        Wq3 = Wq.rearrange("(ko p) n -> p ko n", p=128)
        Wk3 = Wk.rearrange("(ko p) n -> p ko n", p=128)
        Wv3 = Wv.rearrange("(ko p) n -> p ko n", p=128)
        Wo3 = Wo.rearrange("(ko p) n -> p ko n", p=128)

        for _rep in range(repeat):
          for c in range(nchunks):
            b0 = c * CHUNK
            # chunk-resident activation inputs [128, KO, CHUNK] bf16
            qT_sb = qkvT_pool.tile([128, KO, CHUNK], BF16, tag="qT")
            nc.sync.dma_start(out=qT_sb, in_=qT3[:, :, b0:b0 + CHUNK])
            kT_sb = qkvT_pool.tile([128, KO, CHUNK], BF16, tag="kT")
            nc.sync.dma_start(out=kT_sb, in_=kT3[:, :, b0:b0 + CHUNK])
            vT_sb = qkvT_pool.tile([128, KO, CHUNK], BF16, tag="vT")
            nc.sync.dma_start(out=vT_sb, in_=vT3[:, :, b0:b0 + CHUNK])

            # chunk-lifetime buffers
            # qnT/knT/vhT use interleaved [hd, blk, h, s] layout (b = blk*8+s)
            # so a per-block slice [:, blk] is one contiguous 128-wide free dim
            # as required by matmul operands.
            NBLK = CHUNK // SBLK
            qp_sb = chunk_pool.tile([128, NBT, D], F32, tag="qp")      # residual (becomes x)
            qnT_sb = chunk_pool.tile([128, NBLK, H, SBLK], BF16, tag="qnT")
            knT_sb = chunk_pool.tile([128, NBLK, H, SBLK], BF16, tag="knT")
            vhT_sb = chunk_pool.tile([128, NBLK, H, SBLK], BF16, tag="vhT")
            oT_sb = chunk_pool.tile([128, H, CHUNK], BF16, tag="oT")

            # ---- q,k projections (natural space) + normalize + transpose ----
            for ng in range(NG):
                n0 = ng * 512
                for (xT_sb, W3, b_sb, is_q) in (
                    (qT_sb, Wq3, bq_sb, True),
                    (kT_sb, Wk3, bk_sb, False),
                ):
                    ps_list = [proj_psum.tile([128, 512], F32, tag="pp",
                                               name=f"pp_{c}_{ng}_{is_q}_{bt}")
                               for bt in range(NBT)]
                    for ko in range(KO):
                        w_sb = wko_pool.tile([128, 512], BF16, tag="w")
                        nc.scalar.dma_start(out=w_sb, in_=W3[:, ko, n0:n0 + 512])
                        for bt in range(NBT):
                            nc.tensor.matmul(
                                ps_list[bt],
                                xT_sb[:, ko, bt * 128:(bt + 1) * 128],
                                w_sb, start=(ko == 0), stop=False)
                    for bt in range(NBT):
                        ps = ps_list[bt]
                        # bias via K=1 ones-row matmul (broadcast along partitions)
                        nc.tensor.matmul(ps, ones_sb, b_sb[:, n0:n0 + 512],
                                         start=False, stop=True)
                        if is_q:
                            # keep fp32 residual
                            nc.scalar.copy(out=qp_sb[:, bt, n0:n0 + 512], in_=ps)
                        # per-head 1/||.|| for the 4 heads in this n-chunk
                        # (single-input ACT Square + accumulate; DVE can't
                        # read the same PSUM tile twice)
                        rr = small_pool.tile([128, 4], F32, tag="rr")
                        scratch = trans_pool.tile([128, 128], BF16, tag="scr")
                        for h4 in range(4):
                            nc.scalar.activation(
                                out=scratch,
                                in_=ps[:, h4 * 128:(h4 + 1) * 128],
                                func=mybir.ActivationFunctionType.Square,
                                accum_out=rr[:, h4:h4 + 1])
                        nc.scalar.activation(out=rr, in_=rr,
                                             func=mybir.ActivationFunctionType.Sqrt)
                        nc.vector.reciprocal(out=rr, in_=rr)
                        # normalized bf16 copy (natural layout)
                        nrm = trans_pool.tile([128, 512], BF16, tag="nrm")
                        for h4 in range(4):
                            nc.vector.tensor_scalar_mul(
                                out=nrm[:, h4 * 128:(h4 + 1) * 128],
                                in0=ps[:, h4 * 128:(h4 + 1) * 128],
                                scalar1=rr[:, h4:h4 + 1])
                        # transpose each head block -> [hd, b], scatter into
                        # interleaved [hd, blk, h, s] layout
                        dstT = qnT_sb if is_q else knT_sb
                        for h4 in range(4):
                            tp = att_psum.tile([128, 128], BF16, tag="ap")
                            nc.tensor.transpose(
                                tp, nrm[:, h4 * 128:(h4 + 1) * 128], ident_sb)
                            nc.scalar.copy(
                                out=dstT[:, bt * 16:(bt + 1) * 16, ng * 4 + h4, :],
                                in_=tp[:, :].rearrange(
                                    "p (blk s) -> p blk s", s=SBLK))

            # ---- v projection (transposed space) -> vhT ----
            for g in range(H):
                ps_v = [proj_psum.tile([128, 256], F32, tag="pp",
                                        name=f"pv_{c}_{g}_{half}")
                        for half in range(CHUNK // 256)]
                for ko in range(KO):
                    w_sb = wko_pool.tile([128, 128], BF16, tag="wv")
                    nc.scalar.dma_start(out=w_sb,
                                        in_=Wv3[:, ko, g * 128:(g + 1) * 128])
                    for half in range(CHUNK // 256):
                        nc.tensor.matmul(
                            ps_v[half], w_sb,
                            vT_sb[:, ko, half * 256:(half + 1) * 256],
                            start=(ko == 0), stop=(ko == KO - 1))
                for half in range(CHUNK // 256):
                    # add per-partition bias while copying psum->sbuf bf16
                    nc.scalar.activation(
                        out=vhT_sb[:, half * 32:(half + 1) * 32, g, :],
                        in_=ps_v[half][:, :].rearrange(
                            "p (blk s) -> p blk s", s=SBLK),
                        func=mybir.ActivationFunctionType.Identity,
                        bias=bv_sb[:, g:g + 1], scale=1.0)

            # ---- attention: scores + o, 8 samples per matmul ----
            for blk in range(CHUNK // SBLK):
                s0 = blk * SBLK
                # ST[(g,s),(h,s')] = knT_s8.T @ qnT_s8   (contraction over hd)
                st_ps = att_psum.tile([128, 128], F32, tag="ap")
                nc.tensor.matmul(
                    st_ps,
                    knT_sb[:, blk].rearrange("p h s -> p (h s)"),
                    qnT_sb[:, blk].rearrange("p h s -> p (h s)"),
                    start=True, stop=True)
                st_sb = trans_pool.tile([128, 128], BF16, tag="st")
                nc.vector.tensor_mul(out=st_sb, in0=st_ps, in1=mask_sb)
                # V_blk[(g,s),hd] = transpose(vhT[:, blk])
                vb_ps = att_psum.tile([128, 128], BF16, tag="ap")
                nc.tensor.transpose(
                    vb_ps, vhT_sb[:, blk].rearrange("p h s -> p (h s)"), ident_sb)
                vb_sb = trans_pool.tile([128, 128], BF16, tag="vb")
                nc.scalar.copy(out=vb_sb, in_=vb_ps)
                # oT[hd,(h,s)] = V_blk.T @ ST_masked
                o_ps = att_psum.tile([128, 128], F32, tag="ap")
                nc.tensor.matmul(o_ps, vb_sb, st_sb, start=True, stop=True)
                nc.scalar.copy(
                    out=oT_sb[:, :, s0:s0 + SBLK],
                    in_=o_ps[:, :].rearrange("p (h s) -> p h s", h=H))

            # ---- output projection + residual + layernorm ----
            for ng in range(NG):
                n0 = ng * 512
                ps_list = [proj_psum.tile([128, 512], F32, tag="pp",
                                           name=f"po_{c}_{ng}_{bt}")
                           for bt in range(NBT)]
                for h in range(H):
                    w_sb = wko_pool.tile([128, 512], BF16, tag="w")
                    nc.scalar.dma_start(out=w_sb, in_=Wo3[:, h, n0:n0 + 512])
                    for bt in range(NBT):
                        nc.tensor.matmul(
                            ps_list[bt], oT_sb[:, h, bt * 128:(bt + 1) * 128],
                            w_sb, start=(h == 0), stop=False)
                for bt in range(NBT):
                    ps = ps_list[bt]
                    nc.tensor.matmul(ps, ones_sb, bo_sb[:, n0:n0 + 512],
                                     start=False, stop=True)
                    # x = qp + o2 (in place into qp_sb)
                    nc.vector.tensor_add(
                        out=qp_sb[:, bt, n0:n0 + 512],
                        in0=qp_sb[:, bt, n0:n0 + 512], in1=ps)

            for bt in range(NBT):
                x_ap = qp_sb[:, bt, :]
                stats = small_pool.tile([128, 4, 6], F32, tag="bn")
                for sg in range(4):
                    nc.vector.bn_stats(out=stats[:, sg, :],
                                       in_=x_ap[:, sg * 512:(sg + 1) * 512])
                mv = small_pool.tile([128, 2], F32, tag="mv")
                nc.vector.bn_aggr(out=mv, in_=stats)
                rstd = small_pool.tile([128, 1], F32, tag="rstd")
                nc.scalar.activation(out=rstd, in_=mv[:, 1:2],
                                     func=mybir.ActivationFunctionType.Sqrt,
                                     bias=eps_sb, scale=1.0)
                nc.vector.reciprocal(out=rstd, in_=rstd)
                for ng in range(NG):
                    n0 = ng * 512
                    ot = out_pool.tile([128, 512], F32, tag="ot")
                    # (x - mu) * rstd
                    nc.vector.tensor_scalar(
                        out=ot, in0=x_ap[:, n0:n0 + 512],
                        scalar1=mv[:, 0:1], scalar2=rstd,
                        op0=mybir.AluOpType.subtract,
                        op1=mybir.AluOpType.mult)
                    # * gamma
                    nc.vector.tensor_mul(out=ot, in0=ot,
                                         in1=gamma_sb[:, n0:n0 + 512])
                    # + beta
                    nc.gpsimd.tensor_add(out=ot, in0=ot,
                                         in1=beta_sb[:, n0:n0 + 512])
                    nc.sync.dma_start(
                        out=out[b0 + bt * 128:b0 + (bt + 1) * 128, n0:n0 + 512],
                        in_=ot)

    nc.compile()
    return nc


def _prep_host_inputs(q, k, v, Wq, bq, Wk, bk, Wv, bv, Wo, bo, gamma, beta):
    bf = ml_dtypes.bfloat16
    qT = np.ascontiguousarray(q.T).astype(bf)
    kT = np.ascontiguousarray(k.T).astype(bf)
    vT = np.ascontiguousarray(v.T).astype(bf)
    # Wo' row h*128+hd  <- Wo row hd*16+h
    hh, dd = np.divmod(np.arange(D), HD)     # d' = h*HD+hd -> h=hh, hd=dd
    src = dd * H + hh
    Wo_p = np.ascontiguousarray(Wo[src, :]).astype(bf)
    # block-diag mask, 1/HD on (r,c) where r%8 == c%8
    r = np.arange(128)
    m = (r[:, None] % SBLK == r[None, :] % SBLK).astype(np.float32) / HD
    shared = {
        "Wq": np.ascontiguousarray(Wq).astype(bf),
        "Wk": np.ascontiguousarray(Wk).astype(bf),
        "Wv": np.ascontiguousarray(Wv).astype(bf),
        "Wo": Wo_p,
        "bq": bq.reshape(1, D).astype(bf),
        "bk": bk.reshape(1, D).astype(bf),
        "bo": bo.reshape(1, D).astype(bf),
        "bv": np.ascontiguousarray(bv.reshape(KO, 128).T).astype(np.float32),
        "gamma": gamma.reshape(1, D).astype(np.float32),
        "beta": beta.reshape(1, D).astype(np.float32),
        "ident": np.eye(128, dtype=bf),
        "mask": m.astype(bf),
        "ones": np.ones((1, 128), dtype=bf),
    }
    return qT, kT, vT, shared


def kernel(q, k, v, Wq, bq, Wk, bk, Wv, bv, Wo, bo, gamma, beta, _bl=None,
           _ncores=None, _trace=False, _tmpdir=None):
    ncores = _ncores or NCORES
    bl = _bl or (q.shape[0] // ncores)
    qT, kT, vT, shared = _prep_host_inputs(
        q, k, v, Wq, bq, Wk, bk, Wv, bv, Wo, bo, gamma, beta)
    nc = build_bass(bl)
    in_maps = []
    for c in range(ncores):
        m = dict(shared)
        s = slice(c * bl, (c + 1) * bl)
        m["qT"] = np.ascontiguousarray(qT[:, s])
        m["kT"] = np.ascontiguousarray(kT[:, s])
        m["vT"] = np.ascontiguousarray(vT[:, s])
        in_maps.append(m)
    res = run_bass_kernel_spmd(nc, in_maps, core_ids=list(range(ncores)),
                               trace=_trace, tmpdir=_tmpdir)
    outs = [r["out"] for r in res.results]
    full = np.concatenate(outs, axis=0)
    if _trace:
        kernel.last_results = res
    return full.astype(np.float32)



# revision 9
# speedup vs baseline: 24.8416x; 1.4916x over previous
"""MultiHeadAttention (cosine-sim, no softmax) + residual + LayerNorm on 8 TRN2 cores.

Reference math (per sample row x of q/k/v, D=2048, H=16, HD=128):
  qp = q @ Wq + bq   (kept as residual)
  kp = k @ Wk + bk ; vp = v @ Wv + bv
  per head h: qn = qh/||qh||, kn = kh/||kh||
  s[h,g] = (qn_h . kn_g) / HD          # [16,16] per sample
  o[h] = sum_g s[h,g] * vh_g           # [16,128]
  o_flat[hd*16+h] = o[h,hd]            # interleaved flatten
  o2 = o_flat @ Wo + bo
  x = qp + o2 ; out = layernorm(x) * gamma + beta

Sharding: pure data-parallel over batch (4096 rows/core), weights replicated.

Device strategy (per core): fully transposed pipeline, bf16 matmuls with
fp32 PSUM accumulation.
  - All activations live as [feature, sample] tiles; weights are the
    stationary matmul operand (host-packed so every weight DMA is a
    contiguous 512KB transfer).
  - q/k/v projections per head: psum[128 hd, 512 b] = sum_ko
    W[ko,h].T @ xT[ko]; bias folded in as a K=1 matmul.
  - Per-head norm: ACT Square -> ones[128,128] matmul (reduce+broadcast
    in one op) -> ACT Sqrt -> DVE fast reciprocal -> DVE multiply writing
    straight into the interleaved [hd, blk, h, s] attention layout.
  - Attention, 8 samples per matmul: ST[(g,s),(h,s')] = knT8.T @ qnT8,
    masked by a block-diagonal 1/HD constant; V8 = PE-transpose of vhT8;
    oT[hd,(h,s)] = V8.T @ ST_masked.
  - Output projection transposed: o2T[nb-block] = sum_h Wo'[h,nb].T @
    oT[:,h,:]; residual add against qpT (same layout!), so no transpose
    of the residual is ever needed.
  - LayerNorm in transposed space: mean/meansq via two accumulated
    ones-matmuls over the 16 feature blocks; gamma/beta are per-partition
    scalars (GpSimd); final normalized bf16 tiles are PE-transposed back
    to natural [sample, feature] for contiguous f32 output DMA.
"""

from contextlib import ExitStack

import numpy as np
import ml_dtypes

import concourse.bass as bass
import concourse.bacc as bacc
import concourse.mybir as mybir
import concourse.tile as tile
from concourse.bass_utils import run_bass_kernel_spmd

BF16 = mybir.dt.bfloat16
F32 = mybir.dt.float32
AF = mybir.ActivationFunctionType

B, D, H, HD = 32768, 2048, 16, 128
NCORES = 8
EPS = 1e-5
CHUNK = 512          # samples per chunk
KO = D // 128        # 16 contraction blocks
NB = D // 128        # 16 feature blocks (== heads for the d' = h*128+hd map)
SBLK = 8             # samples per attention block matmul
NBLK = CHUNK // SBLK
NBT = CHUNK // 128


def build_bass(bl):
    nc = bacc.Bacc()
    nch = bl // CHUNK

    qT = nc.dram_tensor("qT", [128, nch, KO, CHUNK], BF16, kind="ExternalInput")
    kT = nc.dram_tensor("kT", [128, nch, KO, CHUNK], BF16, kind="ExternalInput")
    vT = nc.dram_tensor("vT", [128, nch, KO, CHUNK], BF16, kind="ExternalInput")
    # weight packs: [p, h, ko, 128] so a per-head slice is contiguous
    wq = nc.dram_tensor("wq", [128, H, KO, 128], BF16, kind="ExternalInput")
    wk = nc.dram_tensor("wk", [128, H, KO, 128], BF16, kind="ExternalInput")
    wv = nc.dram_tensor("wv", [128, H, KO, 128], BF16, kind="ExternalInput")
    # Wo': permuted rows (h*128+hd <- hd*16+h), packed [p=hd, nb, h, 128]
    wo = nc.dram_tensor("wo", [128, NB, H, 128], BF16, kind="ExternalInput")
    bqr = nc.dram_tensor("bqr", [1, D], BF16, kind="ExternalInput")
    bkr = nc.dram_tensor("bkr", [1, D], BF16, kind="ExternalInput")
    bvc = nc.dram_tensor("bvc", [128, H], F32, kind="ExternalInput")
    bor = nc.dram_tensor("bor", [1, D], BF16, kind="ExternalInput")
    gpk = nc.dram_tensor("gpk", [128, NB], F32, kind="ExternalInput")   # gamma
    bpk = nc.dram_tensor("bpk", [128, NB], F32, kind="ExternalInput")   # beta
    ident = nc.dram_tensor("ident", [128, 128], BF16, kind="ExternalInput")
    mask = nc.dram_tensor("mask", [128, 128], BF16, kind="ExternalInput")
    ones128 = nc.dram_tensor("ones128", [128, 128], BF16, kind="ExternalInput")
    ones1 = nc.dram_tensor("ones1", [1, CHUNK], BF16, kind="ExternalInput")
    out = nc.dram_tensor("out", [bl, D], F32, kind="ExternalOutput")

    with tile.TileContext(nc) as tc, ExitStack() as ctx:
        consts = ctx.enter_context(tc.tile_pool(name="consts", bufs=1))
        qin = ctx.enter_context(tc.tile_pool(name="qin", bufs=1))
        kin = ctx.enter_context(tc.tile_pool(name="kin", bufs=1))
        vin = ctx.enter_context(tc.tile_pool(name="vin", bufs=1))
        wpool = ctx.enter_context(tc.tile_pool(name="wpool", bufs=3))
        qpT_pool = ctx.enter_context(tc.tile_pool(name="qpT", bufs=2))
        sq_pool = ctx.enter_context(tc.tile_pool(name="sq", bufs=2))
        nrm_pool = ctx.enter_context(tc.tile_pool(name="nrm", bufs=1))
        rs_pool = ctx.enter_context(tc.tile_pool(name="rs", bufs=2))
        nTpool = ctx.enter_context(tc.tile_pool(name="nT", bufs=1))
        oT_pool = ctx.enter_context(tc.tile_pool(name="oT", bufs=1))
        att_sb = ctx.enter_context(tc.tile_pool(name="att_sb", bufs=2))
        ln_pool = ctx.enter_context(tc.tile_pool(name="ln", bufs=2))
        og_pool = ctx.enter_context(tc.tile_pool(name="og", bufs=4))
        pp = ctx.enter_context(tc.tile_pool(name="pp", bufs=3, space="PSUM"))
        ssp = ctx.enter_context(tc.tile_pool(name="ssp", bufs=2, space="PSUM"))
        att = ctx.enter_context(tc.tile_pool(name="att", bufs=2, space="PSUM"))

        # ---- constants ----
        ident_sb = consts.tile([128, 128], BF16)
        nc.sync.dma_start(out=ident_sb, in_=ident[:, :])
        mask_sb = consts.tile([128, 128], BF16)
        nc.sync.dma_start(out=mask_sb, in_=mask[:, :])
        ones128_sb = consts.tile([128, 128], BF16)
        nc.sync.dma_start(out=ones128_sb, in_=ones128[:, :])
        ones1_sb = consts.tile([1, CHUNK], BF16)
        nc.sync.dma_start(out=ones1_sb, in_=ones1[:, :])
        bq_sb = consts.tile([1, D], BF16)
        nc.sync.dma_start(out=bq_sb, in_=bqr[:, :])
        bk_sb = consts.tile([1, D], BF16)
        nc.sync.dma_start(out=bk_sb, in_=bkr[:, :])
        bvc_sb = consts.tile([128, H], F32)
        nc.sync.dma_start(out=bvc_sb, in_=bvc[:, :])
        bo_sb = consts.tile([1, D], BF16)
        nc.sync.dma_start(out=bo_sb, in_=bor[:, :])
        g_sb = consts.tile([128, NB], F32)
        nc.sync.dma_start(out=g_sb, in_=gpk[:, :])
        b_sb = consts.tile([128, NB], F32)
        nc.sync.dma_start(out=b_sb, in_=bpk[:, :])
        eps_sb = consts.tile([128, 1], F32)
        nc.vector.memset(eps_sb, EPS)

        for c in range(nch):
            qT_sb = qin.tile([128, KO, CHUNK], BF16, tag="qT")
            nc.sync.dma_start(out=qT_sb, in_=qT[:, c])
            kT_sb = kin.tile([128, KO, CHUNK], BF16, tag="kT")
            nc.sync.dma_start(out=kT_sb, in_=kT[:, c])
            vT_sb = vin.tile([128, KO, CHUNK], BF16, tag="vT")
            nc.sync.dma_start(out=vT_sb, in_=vT[:, c])

            # chunk-lifetime interleaved activations [hd, blk, h, s]
            qnT_sb = nTpool.tile([128, NBLK, H, SBLK], BF16, tag="qnT")
            knT_sb = nTpool.tile([128, NBLK, H, SBLK], BF16, tag="knT")
            vhT_sb = nTpool.tile([128, NBLK, H, SBLK], BF16, tag="vhT")
            oT_sb = oT_pool.tile([128, H, CHUNK], BF16, tag="oT")
            # per-head transposed residual qpT; becomes xT after the
            # in-place residual add in stage 4
            qpT_sb = qpT_pool.tile([128, H, CHUNK], BF16, tag="qpT")

            # ---- stage 1: q,k transposed projections + normalize ----
            for h in range(H):
                for (xsb, wd, brow, is_q) in ((qT_sb, wq, bq_sb, True),
                                              (kT_sb, wk, bk_sb, False)):
                    wt = wpool.tile([128, KO, 128], BF16, tag="w")
                    nc.sync.dma_start(out=wt, in_=wd[:, h])
                    ps = pp.tile([128, CHUNK], F32, tag="pp",
                                 name=f"p{'q' if is_q else 'k'}_{c}_{h}")
                    nc.tensor.matmul(ps, brow[:, h * 128:(h + 1) * 128],
                                     ones1_sb, start=True, stop=False)
                    for ko in range(KO):
                        nc.tensor.matmul(ps, wt[:, ko], xsb[:, ko],
                                         start=False, stop=(ko == KO - 1))
                    if is_q:
                        nc.scalar.copy(out=qpT_sb[:, h, :], in_=ps)
                    sq = sq_pool.tile([128, CHUNK], BF16, tag="sq")
                    nc.scalar.activation(out=sq, in_=ps, func=AF.Square)
                    ssb = ssp.tile([128, CHUNK], F32, tag="ss",
                                   name=f"ss_{c}_{h}_{is_q}")
                    nc.tensor.matmul(ssb, ones128_sb, sq, start=True, stop=True)
                    nrm = nrm_pool.tile([128, CHUNK], F32, tag="nrm")
                    nc.scalar.activation(out=nrm, in_=ssb, func=AF.Sqrt)
                    rs = rs_pool.tile([128, CHUNK], F32, tag="rs")
                    nc.vector.reciprocal_approx_fast(out=rs, in_=nrm)
                    dst = qnT_sb if is_q else knT_sb
                    nc.vector.tensor_mul(
                        out=dst[:, :, h, :],
                        in0=ps.rearrange("p (blk s) -> p blk s", s=SBLK),
                        in1=rs.rearrange("p (blk s) -> p blk s", s=SBLK))

            # ---- stage 2: v transposed projection ----
            for h in range(H):
                wt = wpool.tile([128, KO, 128], BF16, tag="w")
                nc.sync.dma_start(out=wt, in_=wv[:, h])
                ps = pp.tile([128, CHUNK], F32, tag="pp", name=f"pv_{c}_{h}")
                for ko in range(KO):
                    nc.tensor.matmul(ps, wt[:, ko], vT_sb[:, ko],
                                     start=(ko == 0), stop=(ko == KO - 1))
                nc.scalar.activation(
                    out=vhT_sb[:, :, h, :],
                    in_=ps.rearrange("p (blk s) -> p blk s", s=SBLK),
                    func=AF.Identity, bias=bvc_sb[:, h:h + 1])

            # ---- stage 3: attention, 8 samples per matmul ----
            for blk in range(NBLK):
                s0 = blk * SBLK
                st_ps = att.tile([128, 128], F32, tag="att")
                nc.tensor.matmul(
                    st_ps, knT_sb[:, blk].rearrange("p h s -> p (h s)"),
                    qnT_sb[:, blk].rearrange("p h s -> p (h s)"),
                    start=True, stop=True)
                st_sb = att_sb.tile([128, 128], BF16, tag="st")
                nc.vector.tensor_mul(out=st_sb, in0=st_ps, in1=mask_sb)
                vb_ps = att.tile([128, 128], BF16, tag="att")
                nc.tensor.transpose(
                    vb_ps, vhT_sb[:, blk].rearrange("p h s -> p (h s)"),
                    ident_sb)
                vb_sb = att_sb.tile([128, 128], BF16, tag="vb")
                nc.vector.tensor_copy(out=vb_sb, in_=vb_ps)
                o_ps = att.tile([128, 128], F32, tag="att")
                nc.tensor.matmul(o_ps, vb_sb, st_sb, start=True, stop=True)
                nc.vector.tensor_copy(
                    out=oT_sb[:, :, s0:s0 + SBLK],
                    in_=o_ps.rearrange("p (h s) -> p h s", h=H))

            # ---- stage 4: transposed output projection + residual ----
            sum_ps = ssp.tile([128, CHUNK], F32, tag="ss", name=f"lsum_{c}")
            sq_ps = ssp.tile([128, CHUNK], F32, tag="ss2", bufs=1,
                             name=f"lsq_{c}")
            for nb in range(NB):
                wt = wpool.tile([128, H, 128], BF16, tag="w")
                nc.sync.dma_start(out=wt, in_=wo[:, nb])
                ps = pp.tile([128, CHUNK], F32, tag="pp", name=f"po_{c}_{nb}")
                nc.tensor.matmul(ps, bo_sb[:, nb * 128:(nb + 1) * 128],
                                 ones1_sb, start=True, stop=False)
                for h in range(H):
                    nc.tensor.matmul(ps, wt[:, h], oT_sb[:, h, :],
                                     start=False, stop=(h == H - 1))
                nc.vector.tensor_add(out=qpT_sb[:, nb, :], in0=ps,
                                     in1=qpT_sb[:, nb, :])
                xsq = sq_pool.tile([128, CHUNK], BF16, tag="sq")
                nc.scalar.activation(out=xsq, in_=qpT_sb[:, nb, :],
                                     func=AF.Square)
                nc.tensor.matmul(sum_ps, ones128_sb, qpT_sb[:, nb, :],
                                 start=(nb == 0), stop=(nb == NB - 1))
                nc.tensor.matmul(sq_ps, ones128_sb, xsq,
                                 start=(nb == 0), stop=(nb == NB - 1))

            # ---- stage 5: LN stats (broadcast rows) ----
            mu = ln_pool.tile([128, CHUNK], BF16, tag="mu")
            nc.scalar.activation(out=mu, in_=sum_ps, func=AF.Copy,
                                 scale=1.0 / D)
            musq = ln_pool.tile([128, CHUNK], F32, tag="musq", bufs=1)
            nc.scalar.activation(out=musq, in_=sum_ps, func=AF.Square,
                                 scale=1.0 / D)
            var = ln_pool.tile([128, CHUNK], F32, tag="var", bufs=1)
            nc.scalar.activation(out=var, in_=sq_ps, func=AF.Copy,
                                 scale=1.0 / D)
            nc.vector.tensor_sub(out=var, in0=var, in1=musq)
            sd = ln_pool.tile([128, CHUNK], F32, tag="sd", bufs=1)
            nc.scalar.activation(out=sd, in_=var, func=AF.Sqrt, bias=eps_sb)
            rstd = ln_pool.tile([128, CHUNK], F32, tag="rstd")
            nc.vector.reciprocal_approx_fast(out=rstd, in_=sd)

            # ---- stage 6: normalize, gamma/beta, transpose, store ----
            b0 = c * CHUNK
            for nb in range(NB):
                d1 = og_pool.tile([128, CHUNK], BF16, tag="d1", bufs=2)
                nc.gpsimd.tensor_sub(out=d1, in0=qpT_sb[:, nb, :], in1=mu)
                nc.gpsimd.tensor_mul(out=d1, in0=d1, in1=rstd)
                d3 = og_pool.tile([128, CHUNK], BF16, tag="d3", bufs=2)
                nc.gpsimd.tensor_scalar(
                    out=d3, in0=d1, scalar1=g_sb[:, nb:nb + 1],
                    scalar2=b_sb[:, nb:nb + 1],
                    op0=mybir.AluOpType.mult, op1=mybir.AluOpType.add)
                for bt in range(NBT):
                    tp = att.tile([128, 128], BF16, tag="att")
                    nc.tensor.transpose(tp, d3[:, bt * 128:(bt + 1) * 128],
                                        ident_sb)
                    og = og_pool.tile([128, 128], F32, tag="og", bufs=3)
                    nc.vector.tensor_copy(out=og, in_=tp)
                    nc.scalar.dma_start(
                        out=out[b0 + bt * 128:b0 + (bt + 1) * 128,
                                nb * 128:(nb + 1) * 128],
                        in_=og)

    nc.compile()
    return nc


def _prep_host_inputs(q, k, v, Wq, bq, Wk, bk, Wv, bv, Wo, bo, gamma, beta,
                      ncores, bl):
    bf = ml_dtypes.bfloat16
    nch = bl // CHUNK

    def pack_xT(x):
        # x [B, D] f32 -> per-core [128, nch, KO, CHUNK] bf16
        xT = np.ascontiguousarray(x.T).astype(bf)          # [D, B]
        view = xT.reshape(KO, 128, ncores, nch, CHUNK)
        return [np.ascontiguousarray(view[:, :, c].transpose(1, 2, 0, 3))
                for c in range(ncores)]

    def pack_w(W):
        # W [D, D] -> [128, H, KO, 128] (lhsT tiles, contiguous per head)
        return np.ascontiguousarray(
            W.reshape(KO, 128, H, 128).transpose(1, 2, 0, 3)).astype(bf)

    # Wo' row h*128+hd <- Wo row hd*16+h, then pack [p=hd, nb, h, 128]
    hh, dd = np.divmod(np.arange(D), HD)
    src = dd * H + hh
    Wo_p = Wo[src, :]
    wo_pack = np.ascontiguousarray(
        Wo_p.reshape(H, 128, NB, 128).transpose(1, 2, 0, 3)).astype(bf)

    r = np.arange(128)
    m = (r[:, None] % SBLK == r[None, :] % SBLK).astype(np.float32) / HD

    shared = {
        "wq": pack_w(Wq), "wk": pack_w(Wk), "wv": pack_w(Wv), "wo": wo_pack,
        "bqr": bq.reshape(1, D).astype(bf),
        "bkr": bk.reshape(1, D).astype(bf),
        "bvc": np.ascontiguousarray(
            bv.reshape(H, 128).T).astype(np.float32),
        "bor": bo.reshape(1, D).astype(bf),
        "gpk": np.ascontiguousarray(
            gamma.reshape(NB, 128).T).astype(np.float32),
        "bpk": np.ascontiguousarray(
            beta.reshape(NB, 128).T).astype(np.float32),
        "ident": np.eye(128, dtype=bf),
        "mask": m.astype(bf),
        "ones128": np.ones((128, 128), dtype=bf),
        "ones1": np.ones((1, CHUNK), dtype=bf),
    }
    return pack_xT(q), pack_xT(k), pack_xT(v), shared


def kernel(q, k, v, Wq, bq, Wk, bk, Wv, bv, Wo, bo, gamma, beta, _bl=None,
           _ncores=None, _trace=False, _tmpdir=None):
    ncores = _ncores or NCORES
    bl = _bl or (q.shape[0] // ncores)
    qTs, kTs, vTs, shared = _prep_host_inputs(
        q, k, v, Wq, bq, Wk, bk, Wv, bv, Wo, bo, gamma, beta, ncores, bl)
    nc = build_bass(bl)
    in_maps = []
    for c in range(ncores):
        m = dict(shared)
        m["qT"] = qTs[c]
        m["kT"] = kTs[c]
        m["vT"] = vTs[c]
        in_maps.append(m)
    res = run_bass_kernel_spmd(nc, in_maps, core_ids=list(range(ncores)),
                               trace=_trace, tmpdir=_tmpdir)
    outs = [r["out"] for r in res.results]
    full = np.concatenate(outs, axis=0)
    if _trace:
        kernel.last_results = res
    return full.astype(np.float32)


# revision 10
# speedup vs baseline: 27.2938x; 1.0987x over previous
"""MultiHeadAttention (cosine-sim, no softmax) + residual + LayerNorm on 8 TRN2 cores.

Reference math (per sample row x of q/k/v, D=2048, H=16, HD=128):
  qp = q @ Wq + bq   (kept as residual)
  kp = k @ Wk + bk ; vp = v @ Wv + bv
  per head h: qn = qh/||qh||, kn = kh/||kh||
  s[h,g] = (qn_h . kn_g) / HD          # [16,16] per sample
  o[h] = sum_g s[h,g] * vh_g           # [16,128]
  o_flat[hd*16+h] = o[h,hd]            # interleaved flatten
  o2 = o_flat @ Wo + bo
  x = qp + o2 ; out = layernorm(x) * gamma + beta

Sharding: pure data-parallel over batch (4096 rows/core), weights replicated.

Device strategy (per core): fully transposed pipeline, bf16 matmuls with
fp32 PSUM accumulation, software-pipelined across 512-sample chunks so the
PE never sits behind the DVE/GpSimd round-trips of the attention/LN phases:

  macro-iteration c emits
    stage 1 (q/k proj+normalize of chunk c)   interleaved with
        attention blocks of chunk c-1 (4 per head group)
    stage 2 (v proj of chunk c)               interleaved with
        o-projection groups, LN stats, normalize+output of chunk c-1

  - Projections per head: psum[128, 512] = sum_ko W[ko,h].T @ xT[ko]
    (weights stationary, host-packed 512KB contiguous DMAs); bias via a
    K=1 matmul.
  - Per-head norm: ACT Square -> ones[128,128] matmul (reduce+broadcast) ->
    ACT Sqrt -> DVE fast reciprocal -> DVE multiply into the interleaved
    [hd, blk, h, s] layout. qnT/knT stored fp8e4 (values in [-1,1]) so both
    chunks' copies fit in SBUF (required for cross-chunk pipelining).
  - Attention per 8 samples: ST[(g,s),(h,s')] = knT8.T @ qnT8, masked by a
    block-diagonal 1/HD constant; V8 = PE-transpose of vhT8;
    oT[hd,(h,s)] = V8.T @ ST_masked.
  - Output projection transposed: o2T[nb] = sum_h Wo'[h,nb].T @ oT[:,h,:];
    residual added in place into qpT (same layout).
  - LayerNorm in transposed space: mean/meansq via two matmul-accumulated
    ones-reductions; gamma/beta as per-partition scalars; normalized bf16
    tiles PE-transposed back to natural layout for contiguous f32 stores.
"""

from contextlib import ExitStack

import numpy as np
import ml_dtypes

import concourse.bass as bass
import concourse.bacc as bacc
import concourse.mybir as mybir
import concourse.tile as tile
from concourse.bass_utils import run_bass_kernel_spmd

BF16 = mybir.dt.bfloat16
F32 = mybir.dt.float32
FP8 = mybir.dt.float8e4
AF = mybir.ActivationFunctionType

B, D, H, HD = 32768, 2048, 16, 128
NCORES = 8
EPS = 1e-5
CHUNK = 512          # samples per chunk
KO = D // 128        # 16 contraction blocks
NB = D // 128        # 16 feature blocks (== heads under d' = h*128+hd)
SBLK = 8             # samples per attention block matmul
NBLK = CHUNK // SBLK
NBT = CHUNK // 128


def build_bass(bl):
    nc = bacc.Bacc()
    nch = bl // CHUNK

    qTd = nc.dram_tensor("qT", [128, nch, KO, CHUNK], BF16, kind="ExternalInput")
    kTd = nc.dram_tensor("kT", [128, nch, KO, CHUNK], BF16, kind="ExternalInput")
    vTd = nc.dram_tensor("vT", [128, nch, KO, CHUNK], BF16, kind="ExternalInput")
    wq = nc.dram_tensor("wq", [128, H, KO, 128], BF16, kind="ExternalInput")
    wk = nc.dram_tensor("wk", [128, H, KO, 128], BF16, kind="ExternalInput")
    wv = nc.dram_tensor("wv", [128, H, KO, 128], BF16, kind="ExternalInput")
    wo = nc.dram_tensor("wo", [128, NB, H, 128], BF16, kind="ExternalInput")
    bqr = nc.dram_tensor("bqr", [1, D], BF16, kind="ExternalInput")
    bkr = nc.dram_tensor("bkr", [1, D], BF16, kind="ExternalInput")
    bvc = nc.dram_tensor("bvc", [128, H], F32, kind="ExternalInput")
    bor = nc.dram_tensor("bor", [1, D], BF16, kind="ExternalInput")
    gpk = nc.dram_tensor("gpk", [128, NB], F32, kind="ExternalInput")
    bpk = nc.dram_tensor("bpk", [128, NB], F32, kind="ExternalInput")
    ident = nc.dram_tensor("ident", [128, 128], BF16, kind="ExternalInput")
    mask = nc.dram_tensor("mask", [128, 128], BF16, kind="ExternalInput")
    ones128 = nc.dram_tensor("ones128", [128, 128], BF16, kind="ExternalInput")
    ones1 = nc.dram_tensor("ones1", [1, CHUNK], BF16, kind="ExternalInput")
    out = nc.dram_tensor("out", [bl, D], F32, kind="ExternalOutput")

    with tile.TileContext(nc) as tc, ExitStack() as ctx:
        consts = ctx.enter_context(tc.tile_pool(name="consts", bufs=1))
        qin = ctx.enter_context(tc.tile_pool(name="qin", bufs=1))
        kin = ctx.enter_context(tc.tile_pool(name="kin", bufs=1))
        vin = ctx.enter_context(tc.tile_pool(name="vin", bufs=1))
        wpool = ctx.enter_context(tc.tile_pool(name="wpool", bufs=3))
        qpT_pool = ctx.enter_context(tc.tile_pool(name="qpT", bufs=2))
        sq_pool = ctx.enter_context(tc.tile_pool(name="sq", bufs=2))
        nrm_pool = ctx.enter_context(tc.tile_pool(name="nrm", bufs=2))
        rs_pool = ctx.enter_context(tc.tile_pool(name="rs", bufs=2))
        nTpool = ctx.enter_context(tc.tile_pool(name="nT", bufs=1))
        oT_pool = ctx.enter_context(tc.tile_pool(name="oT", bufs=1))
        att_sb = ctx.enter_context(tc.tile_pool(name="att_sb", bufs=3))
        ln_pool = ctx.enter_context(tc.tile_pool(name="ln", bufs=2))
        og_pool = ctx.enter_context(tc.tile_pool(name="og", bufs=4))
        pp = ctx.enter_context(tc.tile_pool(name="pp", bufs=2, space="PSUM"))
        ssp = ctx.enter_context(tc.tile_pool(name="ssp", bufs=2, space="PSUM"))
        lnp = ctx.enter_context(tc.tile_pool(name="lnp", bufs=1, space="PSUM"))
        att = ctx.enter_context(tc.tile_pool(name="att", bufs=2, space="PSUM"))

        # ---- constants ----
        ident_sb = consts.tile([128, 128], BF16)
        nc.sync.dma_start(out=ident_sb, in_=ident[:, :])
        mask_sb = consts.tile([128, 128], BF16)
        nc.sync.dma_start(out=mask_sb, in_=mask[:, :])
        ones128_sb = consts.tile([128, 128], BF16)
        nc.sync.dma_start(out=ones128_sb, in_=ones128[:, :])
        ones1_sb = consts.tile([1, CHUNK], BF16)
        nc.sync.dma_start(out=ones1_sb, in_=ones1[:, :])
        bq_sb = consts.tile([1, D], BF16)
        nc.sync.dma_start(out=bq_sb, in_=bqr[:, :])
        bk_sb = consts.tile([1, D], BF16)
        nc.sync.dma_start(out=bk_sb, in_=bkr[:, :])
        bvc_sb = consts.tile([128, H], F32)
        nc.sync.dma_start(out=bvc_sb, in_=bvc[:, :])
        bo_sb = consts.tile([1, D], BF16)
        nc.sync.dma_start(out=bo_sb, in_=bor[:, :])
        g_sb = consts.tile([128, NB], F32)
        nc.sync.dma_start(out=g_sb, in_=gpk[:, :])
        b_sb = consts.tile([128, NB], F32)
        nc.sync.dma_start(out=b_sb, in_=bpk[:, :])
        eps_sb = consts.tile([128, 1], F32)
        nc.vector.memset(eps_sb, EPS)

        st = {}   # per-chunk live tile handles

        def start_chunk(c):
            qT_sb = qin.tile([128, KO, CHUNK], BF16, tag="qT", name=f"qT{c}")
            nc.sync.dma_start(out=qT_sb, in_=qTd[:, c])
            kT_sb = kin.tile([128, KO, CHUNK], BF16, tag="kT", name=f"kT{c}")
            nc.sync.dma_start(out=kT_sb, in_=kTd[:, c])
            vT_sb = vin.tile([128, KO, CHUNK], BF16, tag="vT", name=f"vT{c}")
            nc.sync.dma_start(out=vT_sb, in_=vTd[:, c])
            st[c] = {
                "qT": qT_sb, "kT": kT_sb, "vT": vT_sb,
                "qnT": nTpool.tile([128, NBLK, H, SBLK], FP8, tag="qnT",
                                   bufs=2, name=f"qnT{c}"),
                "knT": nTpool.tile([128, NBLK, H, SBLK], FP8, tag="knT",
                                   bufs=2, name=f"knT{c}"),
                "vhT": nTpool.tile([128, NBLK, H, SBLK], BF16, tag="vhT",
                                   name=f"vhT{c}"),
                "oT": oT_pool.tile([128, H, CHUNK], BF16, tag="oT",
                                   name=f"oT{c}"),
                "qpT": qpT_pool.tile([128, H, CHUNK], BF16, tag="qpT",
                                     name=f"qpT{c}"),
            }

        def emit_qk_head(c, h):
            s = st[c]
            for (xsb, wd, brow, is_q) in ((s["qT"], wq, bq_sb, True),
                                          (s["kT"], wk, bk_sb, False)):
                tag = "q" if is_q else "k"
                wt = wpool.tile([128, KO, 128], BF16, tag="w",
                                name=f"w{tag}_{c}_{h}")
                nc.sync.dma_start(out=wt, in_=wd[:, h])
                ps = pp.tile([128, CHUNK], F32, tag="pp",
                             name=f"p{tag}_{c}_{h}")
                nc.tensor.matmul(ps, brow[:, h * 128:(h + 1) * 128],
                                 ones1_sb, start=True, stop=False)
                for ko in range(KO):
                    nc.tensor.matmul(ps, wt[:, ko], xsb[:, ko],
                                     start=False, stop=(ko == KO - 1))
                if is_q:
                    nc.scalar.copy(out=s["qpT"][:, h, :], in_=ps)
                sq = sq_pool.tile([128, CHUNK], BF16, tag="sq",
                                  name=f"sq_{c}_{h}_{tag}")
                nc.scalar.activation(out=sq, in_=ps, func=AF.Square)
                ssb = ssp.tile([128, CHUNK], F32, tag="ss",
                               name=f"ss_{c}_{h}_{tag}")
                nc.tensor.matmul(ssb, ones128_sb, sq, start=True, stop=True)
                nrm = nrm_pool.tile([128, CHUNK], F32, tag="nrm",
                                    name=f"nrm_{c}_{h}_{tag}")
                nc.scalar.activation(out=nrm, in_=ssb, func=AF.Sqrt)
                rs = rs_pool.tile([128, CHUNK], F32, tag="rs",
                                  name=f"rs_{c}_{h}_{tag}")
                nc.vector.reciprocal_approx_fast(out=rs, in_=nrm)
                dst = s["qnT"] if is_q else s["knT"]
                nc.vector.tensor_mul(
                    out=dst[:, :, h, :],
                    in0=ps.rearrange("p (blk s) -> p blk s", s=SBLK),
                    in1=rs.rearrange("p (blk s) -> p blk s", s=SBLK))

        def emit_v_head(c, h):
            s = st[c]
            wt = wpool.tile([128, KO, 128], BF16, tag="w", name=f"wv_{c}_{h}")
            nc.sync.dma_start(out=wt, in_=wv[:, h])
            ps = pp.tile([128, CHUNK], F32, tag="pp", name=f"pv_{c}_{h}")
            for ko in range(KO):
                nc.tensor.matmul(ps, wt[:, ko], s["vT"][:, ko],
                                 start=(ko == 0), stop=(ko == KO - 1))
            nc.scalar.activation(
                out=s["vhT"][:, :, h, :],
                in_=ps.rearrange("p (blk s) -> p blk s", s=SBLK),
                func=AF.Identity, bias=bvc_sb[:, h:h + 1])

        def emit_att_block(c, blk):
            s = st[c]
            s0 = blk * SBLK
            st_ps = att.tile([128, 128], F32, tag="att", name=f"st_{c}_{blk}")
            nc.tensor.matmul(
                st_ps, s["knT"][:, blk].rearrange("p h s -> p (h s)"),
                s["qnT"][:, blk].rearrange("p h s -> p (h s)"),
                start=True, stop=True)
            st_t = att_sb.tile([128, 128], BF16, tag="st",
                               name=f"stb_{c}_{blk}")
            nc.vector.tensor_mul(out=st_t, in0=st_ps, in1=mask_sb)
            vb_ps = att.tile([128, 128], BF16, tag="att", name=f"vb_{c}_{blk}")
            nc.tensor.transpose(
                vb_ps, s["vhT"][:, blk].rearrange("p h s -> p (h s)"),
                ident_sb)
            vb_sb = att_sb.tile([128, 128], BF16, tag="vb",
                                name=f"vbs_{c}_{blk}")
            nc.scalar.copy(out=vb_sb, in_=vb_ps)
            o_ps = att.tile([128, 128], F32, tag="att", name=f"o_{c}_{blk}")
            nc.tensor.matmul(o_ps, vb_sb, st_t, start=True, stop=True)
            nc.vector.tensor_copy(
                out=s["oT"][:, :, s0:s0 + SBLK],
                in_=o_ps.rearrange("p (h s) -> p h s", h=H))

        def emit_oproj_nb(c, nb):
            s = st[c]
            if nb == 0:
                s["sum_ps"] = lnp.tile([128, CHUNK], F32, tag="lnsum",
                                       name=f"lsum_{c}")
                s["sq_ps"] = lnp.tile([128, CHUNK], F32, tag="lnsq",
                                      name=f"lsq_{c}")
            wt = wpool.tile([128, H, 128], BF16, tag="w", name=f"wo_{c}_{nb}")
            nc.sync.dma_start(out=wt, in_=wo[:, nb])
            ps = pp.tile([128, CHUNK], F32, tag="pp", name=f"po_{c}_{nb}")
            nc.tensor.matmul(ps, bo_sb[:, nb * 128:(nb + 1) * 128],
                             ones1_sb, start=True, stop=False)
            for h in range(H):
                nc.tensor.matmul(ps, wt[:, h], s["oT"][:, h, :],
                                 start=False, stop=(h == H - 1))
            nc.vector.tensor_add(out=s["qpT"][:, nb, :], in0=ps,
                                 in1=s["qpT"][:, nb, :])
            xsq = sq_pool.tile([128, CHUNK], BF16, tag="sq",
                               name=f"xsq_{c}_{nb}")
            nc.scalar.activation(out=xsq, in_=s["qpT"][:, nb, :],
                                 func=AF.Square)
            nc.tensor.matmul(s["sum_ps"], ones128_sb, s["qpT"][:, nb, :],
                             start=(nb == 0), stop=(nb == NB - 1))
            nc.tensor.matmul(s["sq_ps"], ones128_sb, xsq,
                             start=(nb == 0), stop=(nb == NB - 1))

        def emit_ln_stats(c):
            s = st[c]
            mu = ln_pool.tile([128, CHUNK], BF16, tag="mu", name=f"mu_{c}")
            nc.scalar.activation(out=mu, in_=s["sum_ps"], func=AF.Copy,
                                 scale=1.0 / D)
            musq = ln_pool.tile([128, CHUNK], F32, tag="musq", bufs=1,
                                name=f"musq_{c}")
            nc.scalar.activation(out=musq, in_=s["sum_ps"], func=AF.Square,
                                 scale=1.0 / D)
            var = ln_pool.tile([128, CHUNK], F32, tag="var", bufs=1,
                               name=f"var_{c}")
            nc.scalar.activation(out=var, in_=s["sq_ps"], func=AF.Copy,
                                 scale=1.0 / D)
            nc.vector.tensor_sub(out=var, in0=var, in1=musq)
            sd = ln_pool.tile([128, CHUNK], F32, tag="sd", bufs=1,
                              name=f"sd_{c}")
            nc.scalar.activation(out=sd, in_=var, func=AF.Sqrt, bias=eps_sb)
            rstd = ln_pool.tile([128, CHUNK], F32, tag="rstd",
                                name=f"rstd_{c}")
            nc.vector.reciprocal_approx_fast(out=rstd, in_=sd)
            s["mu"], s["rstd"] = mu, rstd

        def emit_out_nb(c, nb):
            s = st[c]
            b0 = c * CHUNK
            d1 = og_pool.tile([128, CHUNK], BF16, tag="d1", bufs=2,
                              name=f"d1_{c}_{nb}")
            nc.vector.tensor_sub(out=d1, in0=s["qpT"][:, nb, :], in1=s["mu"])
            nc.gpsimd.tensor_mul(out=d1, in0=d1, in1=s["rstd"])
            d3 = og_pool.tile([128, CHUNK], BF16, tag="d3", bufs=2,
                              name=f"d3_{c}_{nb}")
            nc.vector.tensor_scalar(
                out=d3, in0=d1, scalar1=g_sb[:, nb:nb + 1],
                scalar2=b_sb[:, nb:nb + 1],
                op0=mybir.AluOpType.mult, op1=mybir.AluOpType.add)
            for bt in range(NBT):
                tp = att.tile([128, 128], BF16, tag="att",
                              name=f"tp_{c}_{nb}_{bt}")
                nc.tensor.transpose(tp, d3[:, bt * 128:(bt + 1) * 128],
                                    ident_sb)
                og = og_pool.tile([128, 128], F32, tag="og", bufs=3,
                                  name=f"og_{c}_{nb}_{bt}")
                nc.vector.tensor_copy(out=og, in_=tp)
                nc.scalar.dma_start(
                    out=out[b0 + bt * 128:b0 + (bt + 1) * 128,
                            nb * 128:(nb + 1) * 128],
                    in_=og)

        def emit_macro(c, prev):
            """stage1+2 of chunk c (if any) interleaved with the attention /
            oproj / LN / output work of chunk prev (if any)."""
            if c is not None:
                start_chunk(c)
                for h in range(H):
                    emit_qk_head(c, h)
                    if prev is not None:
                        for blk in range(4 * h, 4 * h + 4):
                            emit_att_block(prev, blk)
                for h in range(H):
                    emit_v_head(c, h)
                    if prev is not None:
                        if h < 8:
                            emit_oproj_nb(prev, 2 * h)
                            emit_oproj_nb(prev, 2 * h + 1)
                        elif h == 8:
                            emit_ln_stats(prev)
                            emit_out_nb(prev, 0)
                            emit_out_nb(prev, 1)
                        else:
                            emit_out_nb(prev, 2 * (h - 8))
                            emit_out_nb(prev, 2 * (h - 8) + 1)
                if prev is not None:
                    for nb in range(2 * (H - 8), NB):
                        emit_out_nb(prev, nb)
            else:
                # flush tail: no next chunk to interleave with
                for blk in range(NBLK):
                    emit_att_block(prev, blk)
                for nb in range(NB):
                    emit_oproj_nb(prev, nb)
                emit_ln_stats(prev)
                for nb in range(NB):
                    emit_out_nb(prev, nb)
            if prev is not None:
                del st[prev]

        for c in range(nch):
            emit_macro(c, c - 1 if c > 0 else None)
        emit_macro(None, nch - 1)

    nc.compile()
    return nc


def _prep_host_inputs(q, k, v, Wq, bq, Wk, bk, Wv, bv, Wo, bo, gamma, beta,
                      ncores, bl):
    bf = ml_dtypes.bfloat16
    nch = bl // CHUNK

    def pack_xT(x):
        xT = np.ascontiguousarray(x.T).astype(bf)          # [D, B]
        view = xT.reshape(KO, 128, ncores, nch, CHUNK)
        return [np.ascontiguousarray(view[:, :, c].transpose(1, 2, 0, 3))
                for c in range(ncores)]

    def pack_w(W):
        return np.ascontiguousarray(
            W.reshape(KO, 128, H, 128).transpose(1, 2, 0, 3)).astype(bf)

    hh, dd = np.divmod(np.arange(D), HD)
    src = dd * H + hh
    Wo_p = Wo[src, :]
    wo_pack = np.ascontiguousarray(
        Wo_p.reshape(H, 128, NB, 128).transpose(1, 2, 0, 3)).astype(bf)

    r = np.arange(128)
    m = (r[:, None] % SBLK == r[None, :] % SBLK).astype(np.float32) / HD

    shared = {
        "wq": pack_w(Wq), "wk": pack_w(Wk), "wv": pack_w(Wv), "wo": wo_pack,
        "bqr": bq.reshape(1, D).astype(bf),
        "bkr": bk.reshape(1, D).astype(bf),
        "bvc": np.ascontiguousarray(
            bv.reshape(H, 128).T).astype(np.float32),
        "bor": bo.reshape(1, D).astype(bf),
        "gpk": np.ascontiguousarray(
            gamma.reshape(NB, 128).T).astype(np.float32),
        "bpk": np.ascontiguousarray(
            beta.reshape(NB, 128).T).astype(np.float32),
        "ident": np.eye(128, dtype=bf),
        "mask": m.astype(bf),
        "ones128": np.ones((128, 128), dtype=bf),
        "ones1": np.ones((1, CHUNK), dtype=bf),
    }
    return pack_xT(q), pack_xT(k), pack_xT(v), shared


def kernel(q, k, v, Wq, bq, Wk, bk, Wv, bv, Wo, bo, gamma, beta, _bl=None,
           _ncores=None, _trace=False, _tmpdir=None):
    ncores = _ncores or NCORES
    bl = _bl or (q.shape[0] // ncores)
    qTs, kTs, vTs, shared = _prep_host_inputs(
        q, k, v, Wq, bq, Wk, bk, Wv, bv, Wo, bo, gamma, beta, ncores, bl)
    nc = build_bass(bl)
    in_maps = []
    for c in range(ncores):
        m = dict(shared)
        m["qT"] = qTs[c]
        m["kT"] = kTs[c]
        m["vT"] = vTs[c]
        in_maps.append(m)
    res = run_bass_kernel_spmd(nc, in_maps, core_ids=list(range(ncores)),
                               trace=_trace, tmpdir=_tmpdir)
    outs = [r["out"] for r in res.results]
    full = np.concatenate(outs, axis=0)
    if _trace:
        kernel.last_results = res
    return full.astype(np.float32)


# revision 13
# speedup vs baseline: 28.6969x; 1.0514x over previous
"""MultiHeadAttention (cosine-sim, no softmax) + residual + LayerNorm on 8 TRN2 cores.

Reference math (per sample row x of q/k/v, D=2048, H=16, HD=128):
  qp = q @ Wq + bq   (kept as residual)
  kp = k @ Wk + bk ; vp = v @ Wv + bv
  per head h: qn = qh/||qh||, kn = kh/||kh||
  s[h,g] = (qn_h . kn_g) / HD          # [16,16] per sample
  o[h] = sum_g s[h,g] * vh_g           # [16,128]
  o_flat[hd*16+h] = o[h,hd]            # interleaved flatten
  o2 = o_flat @ Wo + bo
  x = qp + o2 ; out = layernorm(x) * gamma + beta

Sharding: pure data-parallel over batch (4096 rows/core), weights replicated.

Device strategy (per core): fully transposed pipeline, bf16 matmuls with
fp32 PSUM accumulation, software-pipelined across 512-sample chunks so the
PE never sits behind the DVE/GpSimd round-trips of the attention/LN phases:

  macro-iteration c emits
    stage 1 (q/k proj+normalize of chunk c)   interleaved with
        attention blocks of chunk c-1 (4 per head group)
    stage 2 (v proj of chunk c)               interleaved with
        o-projection groups, LN stats, normalize+output of chunk c-1

  - Projections per head: psum[128, 512] = sum_ko W[ko,h].T @ xT[ko]
    (weights stationary, host-packed 512KB contiguous DMAs); bias via a
    K=1 matmul.
  - Per-head norm: ACT Square -> ones[128,128] matmul (reduce+broadcast) ->
    ACT Sqrt -> DVE fast reciprocal -> DVE multiply into the interleaved
    [hd, blk, h, s] layout. qnT/knT stored fp8e4 (values in [-1,1]) so both
    chunks' copies fit in SBUF (required for cross-chunk pipelining).
  - Attention per 8 samples: ST[(g,s),(h,s')] = knT8.T @ qnT8, masked by a
    block-diagonal 1/HD constant; V8 = PE-transpose of vhT8;
    oT[hd,(h,s)] = V8.T @ ST_masked.
  - Output projection transposed: o2T[nb] = sum_h Wo'[h,nb].T @ oT[:,h,:];
    residual added in place into qpT (same layout).
  - LayerNorm in transposed space: mean/meansq via two matmul-accumulated
    ones-reductions; gamma/beta as per-partition scalars; normalized bf16
    tiles PE-transposed back to natural layout for contiguous f32 stores.
"""

from contextlib import ExitStack

import numpy as np
import ml_dtypes

import concourse.bass as bass
import concourse.bacc as bacc
import concourse.mybir as mybir
import concourse.tile as tile
from concourse.bass_utils import run_bass_kernel_spmd

BF16 = mybir.dt.bfloat16
F32 = mybir.dt.float32
FP8 = mybir.dt.float8e4
AF = mybir.ActivationFunctionType

B, D, H, HD = 32768, 2048, 16, 128
NCORES = 8
EPS = 1e-5
CHUNK = 512          # samples per chunk
KO = D // 128        # 16 contraction blocks
NB = D // 128        # 16 feature blocks (== heads under d' = h*128+hd)
SBLK = 8             # samples per attention block matmul
NBLK = CHUNK // SBLK
NBT = CHUNK // 128


def build_bass(bl):
    nc = bacc.Bacc()
    nch = bl // CHUNK

    qTd = nc.dram_tensor("qT", [128, nch, KO, CHUNK], BF16, kind="ExternalInput")
    kTd = nc.dram_tensor("kT", [128, nch, KO, CHUNK], BF16, kind="ExternalInput")
    vTd = nc.dram_tensor("vT", [128, nch, KO, CHUNK], BF16, kind="ExternalInput")
    wq = nc.dram_tensor("wq", [128, H, KO, 128], BF16, kind="ExternalInput")
    wk = nc.dram_tensor("wk", [128, H, KO, 128], BF16, kind="ExternalInput")
    wv = nc.dram_tensor("wv", [128, H, KO, 128], BF16, kind="ExternalInput")
    wo = nc.dram_tensor("wo", [128, NB, H, 128], BF16, kind="ExternalInput")
    bqr = nc.dram_tensor("bqr", [1, D], BF16, kind="ExternalInput")
    bkr = nc.dram_tensor("bkr", [1, D], BF16, kind="ExternalInput")
    bvc = nc.dram_tensor("bvc", [128, H], F32, kind="ExternalInput")
    bor = nc.dram_tensor("bor", [1, D], BF16, kind="ExternalInput")
    gpk = nc.dram_tensor("gpk", [128, NB], F32, kind="ExternalInput")
    bpk = nc.dram_tensor("bpk", [128, NB], F32, kind="ExternalInput")
    ident = nc.dram_tensor("ident", [128, 128], BF16, kind="ExternalInput")
    mask = nc.dram_tensor("mask", [128, 128], BF16, kind="ExternalInput")
    ones128 = nc.dram_tensor("ones128", [128, 128], BF16, kind="ExternalInput")
    ones1 = nc.dram_tensor("ones1", [1, CHUNK], BF16, kind="ExternalInput")
    out = nc.dram_tensor("out", [bl, D], F32, kind="ExternalOutput")

    with tile.TileContext(nc) as tc, ExitStack() as ctx:
        consts = ctx.enter_context(tc.tile_pool(name="consts", bufs=1))
        qin = ctx.enter_context(tc.tile_pool(name="qin", bufs=1))
        kin = ctx.enter_context(tc.tile_pool(name="kin", bufs=1))
        vin = ctx.enter_context(tc.tile_pool(name="vin", bufs=1))
        wpool = ctx.enter_context(tc.tile_pool(name="wpool", bufs=3))
        qpT_pool = ctx.enter_context(tc.tile_pool(name="qpT", bufs=2))
        sq_pool = ctx.enter_context(tc.tile_pool(name="sq", bufs=2))
        nrm_pool = ctx.enter_context(tc.tile_pool(name="nrm", bufs=2))
        rs_pool = ctx.enter_context(tc.tile_pool(name="rs", bufs=2))
        nTpool = ctx.enter_context(tc.tile_pool(name="nT", bufs=1))
        oT_pool = ctx.enter_context(tc.tile_pool(name="oT", bufs=1))
        att_sb = ctx.enter_context(tc.tile_pool(name="att_sb", bufs=3))
        ln_pool = ctx.enter_context(tc.tile_pool(name="ln", bufs=2))
        og_pool = ctx.enter_context(tc.tile_pool(name="og", bufs=4))
        pp = ctx.enter_context(tc.tile_pool(name="pp", bufs=2, space="PSUM"))
        ssp = ctx.enter_context(tc.tile_pool(name="ssp", bufs=1, space="PSUM"))
        lnp = ctx.enter_context(tc.tile_pool(name="lnp", bufs=1, space="PSUM"))
        att = ctx.enter_context(tc.tile_pool(name="att", bufs=3, space="PSUM"))

        # ---- constants ----
        ident_sb = consts.tile([128, 128], BF16)
        nc.sync.dma_start(out=ident_sb, in_=ident[:, :])
        mask_sb = consts.tile([128, 128], BF16)
        nc.sync.dma_start(out=mask_sb, in_=mask[:, :])
        ones128_sb = consts.tile([128, 128], BF16)
        nc.sync.dma_start(out=ones128_sb, in_=ones128[:, :])
        ones1_sb = consts.tile([1, CHUNK], BF16)
        nc.sync.dma_start(out=ones1_sb, in_=ones1[:, :])
        bq_sb = consts.tile([1, D], BF16)
        nc.sync.dma_start(out=bq_sb, in_=bqr[:, :])
        bk_sb = consts.tile([1, D], BF16)
        nc.sync.dma_start(out=bk_sb, in_=bkr[:, :])
        bvc_sb = consts.tile([128, H], F32)
        nc.sync.dma_start(out=bvc_sb, in_=bvc[:, :])
        bo_sb = consts.tile([1, D], BF16)
        nc.sync.dma_start(out=bo_sb, in_=bor[:, :])
        g_sb = consts.tile([128, NB], F32)
        nc.sync.dma_start(out=g_sb, in_=gpk[:, :])
        b_sb = consts.tile([128, NB], F32)
        nc.sync.dma_start(out=b_sb, in_=bpk[:, :])
        eps_sb = consts.tile([128, 1], F32)
        nc.vector.memset(eps_sb, EPS)

        st = {}   # per-chunk live tile handles
        filler = []   # deferred PE micro-ops (attention / out-transposes)

        def pop_filler(n):
            for _ in range(n):
                if not filler:
                    return
                filler.pop(0)()

        def start_chunk(c):
            qT_sb = qin.tile([128, KO, CHUNK], BF16, tag="qT", name=f"qT{c}")
            nc.sync.dma_start(out=qT_sb, in_=qTd[:, c])
            kT_sb = kin.tile([128, KO, CHUNK], BF16, tag="kT", name=f"kT{c}")
            nc.sync.dma_start(out=kT_sb, in_=kTd[:, c])
            vT_sb = vin.tile([128, KO, CHUNK], BF16, tag="vT", name=f"vT{c}")
            nc.sync.dma_start(out=vT_sb, in_=vTd[:, c])
            st[c] = {
                "qT": qT_sb, "kT": kT_sb, "vT": vT_sb,
                "qnT": nTpool.tile([128, NBLK, H, SBLK], FP8, tag="qnT",
                                   bufs=2, name=f"qnT{c}"),
                "knT": nTpool.tile([128, NBLK, H, SBLK], FP8, tag="knT",
                                   bufs=2, name=f"knT{c}"),
                "vhT": nTpool.tile([128, NBLK, H, SBLK], BF16, tag="vhT",
                                   name=f"vhT{c}"),
                "oT": oT_pool.tile([128, H, CHUNK], BF16, tag="oT",
                                   name=f"oT{c}"),
                "qpT": qpT_pool.tile([128, H, CHUNK], BF16, tag="qpT",
                                     name=f"qpT{c}"),
                "st_t": {}, "vb_sb": {},
            }

        # ---- attention micro-ops (spread through projection streams) ----
        def att_st(c, blk):
            def f():
                s = st[c]
                st_ps = att.tile([128, 128], F32, tag="att",
                                 name=f"st_{c}_{blk}")
                nc.tensor.matmul(
                    st_ps, s["knT"][:, blk].rearrange("p h s -> p (h s)"),
                    s["qnT"][:, blk].rearrange("p h s -> p (h s)"),
                    start=True, stop=True)
                st_t = att_sb.tile([128, 128], BF16, tag="st", bufs=4,
                                   name=f"stb_{c}_{blk}")
                nc.vector.tensor_mul(out=st_t, in0=st_ps, in1=mask_sb)
                s["st_t"][blk] = st_t
            return f

        def att_vb(c, blk):
            def f():
                s = st[c]
                vb_ps = att.tile([128, 128], BF16, tag="att",
                                 name=f"vb_{c}_{blk}")
                nc.tensor.transpose(
                    vb_ps, s["vhT"][:, blk].rearrange("p h s -> p (h s)"),
                    ident_sb)
                vb = att_sb.tile([128, 128], BF16, tag="vb", bufs=4,
                                 name=f"vbs_{c}_{blk}")
                nc.scalar.copy(out=vb, in_=vb_ps)
                s["vb_sb"][blk] = vb
            return f

        def att_o(c, blk):
            def f():
                s = st[c]
                o_ps = att.tile([128, 128], F32, tag="att",
                                name=f"o_{c}_{blk}")
                nc.tensor.matmul(o_ps, s["vb_sb"].pop(blk),
                                 s["st_t"].pop(blk), start=True, stop=True)
                nc.vector.tensor_copy(
                    out=s["oT"][:, :, blk * SBLK:(blk + 1) * SBLK],
                    in_=o_ps.rearrange("p (h s) -> p h s", h=H))
            return f

        def queue_attention(c):
            # block stages pipelined: dependent ops ~3 queue slots apart
            for i in range(NBLK + 2):
                if i < NBLK:
                    filler.append(att_st(c, i))
                if 1 <= i <= NBLK:
                    filler.append(att_vb(c, i - 1))
                if 2 <= i:
                    filler.append(att_o(c, i - 2))

        def out_tp(c, nb, bt, d3):
            def f():
                b0 = c * CHUNK
                tp = att.tile([128, 128], BF16, tag="att",
                              name=f"tp_{c}_{nb}_{bt}")
                nc.tensor.transpose(tp, d3[:, bt * 128:(bt + 1) * 128],
                                    ident_sb)
                og = og_pool.tile([128, 128], F32, tag="og", bufs=3,
                                  name=f"og_{c}_{nb}_{bt}")
                nc.vector.tensor_copy(out=og, in_=tp)
                nc.scalar.dma_start(
                    out=out[b0 + bt * 128:b0 + (bt + 1) * 128,
                            nb * 128:(nb + 1) * 128],
                    in_=og)
            return f

        def emit_qk_head(c, h):
            s = st[c]
            for (xsb, wd, brow, is_q) in ((s["qT"], wq, bq_sb, True),
                                          (s["kT"], wk, bk_sb, False)):
                tag = "q" if is_q else "k"
                wt = wpool.tile([128, KO, 128], BF16, tag="w",
                                name=f"w{tag}_{c}_{h}")
                nc.sync.dma_start(out=wt, in_=wd[:, h])
                ps = pp.tile([128, CHUNK], F32, tag="pp",
                             name=f"p{tag}_{c}_{h}")
                nc.tensor.matmul(ps, brow[:, h * 128:(h + 1) * 128],
                                 ones1_sb, start=True, stop=False)
                for ko in range(KO):
                    nc.tensor.matmul(ps, wt[:, ko], xsb[:, ko],
                                     start=False, stop=(ko == KO - 1))
                    if ko % 4 == 3:
                        pop_filler(2 if ko % 8 == 3 else 1)
                if is_q:
                    nc.scalar.copy(out=s["qpT"][:, h, :], in_=ps)
                sq = sq_pool.tile([128, CHUNK], BF16, tag="sq",
                                  name=f"sq_{c}_{h}_{tag}")
                nc.scalar.activation(out=sq, in_=ps, func=AF.Square)
                ssb = ssp.tile([128, CHUNK], F32, tag="ss",
                               name=f"ss_{c}_{h}_{tag}")
                nc.tensor.matmul(ssb, ones128_sb, sq, start=True, stop=True)
                nrm = nrm_pool.tile([128, CHUNK], F32, tag="nrm",
                                    name=f"nrm_{c}_{h}_{tag}")
                nc.scalar.activation(out=nrm, in_=ssb, func=AF.Sqrt)
                rs = rs_pool.tile([128, CHUNK], F32, tag="rs",
                                  name=f"rs_{c}_{h}_{tag}")
                nc.vector.reciprocal_approx_fast(out=rs, in_=nrm)
                dst = s["qnT"] if is_q else s["knT"]
                nc.vector.tensor_mul(
                    out=dst[:, :, h, :],
                    in0=ps.rearrange("p (blk s) -> p blk s", s=SBLK),
                    in1=rs.rearrange("p (blk s) -> p blk s", s=SBLK))

        def emit_v_head(c, h):
            s = st[c]
            wt = wpool.tile([128, KO, 128], BF16, tag="w", name=f"wv_{c}_{h}")
            nc.sync.dma_start(out=wt, in_=wv[:, h])
            ps = pp.tile([128, CHUNK], F32, tag="pp", name=f"pv_{c}_{h}")
            for ko in range(KO):
                nc.tensor.matmul(ps, wt[:, ko], s["vT"][:, ko],
                                 start=(ko == 0), stop=(ko == KO - 1))
                if ko % 4 == 3:
                    pop_filler(1)
            nc.scalar.activation(
                out=s["vhT"][:, :, h, :],
                in_=ps.rearrange("p (blk s) -> p blk s", s=SBLK),
                func=AF.Identity, bias=bvc_sb[:, h:h + 1])

        def emit_oproj_nb(c, nb, half=None, pops=1):
            s = st[c]
            if nb == 0 and half in (None, 0):
                s["sum_ps"] = lnp.tile([128, CHUNK], F32, tag="lnsum",
                                       name=f"lsum_{c}")
                s["sq_ps"] = lnp.tile([128, CHUNK], F32, tag="lnsq",
                                      name=f"lsq_{c}")
            if half is None:
                cs = slice(0, CHUNK)
                first, last = nb == 0, nb == NB - 1
            else:
                cs = slice(half * (CHUNK // 2), (half + 1) * (CHUNK // 2))
                first = nb == 0 and half == 0
                last = nb == NB - 1 and half == 1
            n = cs.stop - cs.start
            wt = wpool.tile([128, H, 128], BF16, tag="w",
                            name=f"wo_{c}_{nb}_{half}")
            nc.sync.dma_start(out=wt, in_=wo[:, nb])
            ps = pp.tile([128, n], F32, tag="pp", name=f"po_{c}_{nb}_{half}")
            nc.tensor.matmul(ps, bo_sb[:, nb * 128:(nb + 1) * 128],
                             ones1_sb[:, :n], start=True, stop=False)
            for h in range(H):
                nc.tensor.matmul(ps, wt[:, h], s["oT"][:, h, cs],
                                 start=False, stop=(h == H - 1))
                if h % 4 == 3:
                    pop_filler(pops)
            nc.vector.tensor_add(out=s["qpT"][:, nb, cs], in0=ps,
                                 in1=s["qpT"][:, nb, cs])
            xsq = sq_pool.tile([128, n], BF16, tag="sq",
                               name=f"xsq_{c}_{nb}_{half}")
            nc.scalar.activation(out=xsq, in_=s["qpT"][:, nb, cs],
                                 func=AF.Square)
            nc.tensor.matmul(s["sum_ps"][:, cs], ones128_sb,
                             s["qpT"][:, nb, cs],
                             start=first, stop=last)
            nc.tensor.matmul(s["sq_ps"][:, cs], ones128_sb, xsq,
                             start=first, stop=last)

        def emit_ln_stats(c):
            s = st[c]
            mu = ln_pool.tile([128, CHUNK], BF16, tag="mu", name=f"mu_{c}")
            nc.scalar.activation(out=mu, in_=s["sum_ps"], func=AF.Copy,
                                 scale=1.0 / D)
            musq = ln_pool.tile([128, CHUNK], F32, tag="musq", bufs=1,
                                name=f"musq_{c}")
            nc.scalar.activation(out=musq, in_=s["sum_ps"], func=AF.Square,
                                 scale=1.0 / D)
            var = ln_pool.tile([128, CHUNK], F32, tag="var", bufs=1,
                               name=f"var_{c}")
            nc.scalar.activation(out=var, in_=s["sq_ps"], func=AF.Copy,
                                 scale=1.0 / D)
            nc.vector.tensor_sub(out=var, in0=var, in1=musq)
            sd = ln_pool.tile([128, CHUNK], F32, tag="sd", bufs=1,
                              name=f"sd_{c}")
            nc.scalar.activation(out=sd, in_=var, func=AF.Sqrt, bias=eps_sb)
            rstd = ln_pool.tile([128, CHUNK], F32, tag="rstd",
                                name=f"rstd_{c}")
            nc.vector.reciprocal_approx_fast(out=rstd, in_=sd)
            s["mu"], s["rstd"] = mu, rstd

        def emit_out_nb(c, nb, defer=True):
            s = st[c]
            d1 = og_pool.tile([128, CHUNK], BF16, tag="d1", bufs=2,
                              name=f"d1_{c}_{nb}")
            nc.vector.tensor_sub(out=d1, in0=s["qpT"][:, nb, :], in1=s["mu"])
            nc.gpsimd.tensor_mul(out=d1, in0=d1, in1=s["rstd"])
            d3 = og_pool.tile([128, CHUNK], BF16, tag="d3", bufs=3,
                              name=f"d3_{c}_{nb}")
            nc.vector.tensor_scalar(
                out=d3, in0=d1, scalar1=g_sb[:, nb:nb + 1],
                scalar2=b_sb[:, nb:nb + 1],
                op0=mybir.AluOpType.mult, op1=mybir.AluOpType.add)
            for bt in range(NBT):
                f = out_tp(c, nb, bt, d3)
                if defer:
                    filler.append(f)
                else:
                    f()

        def emit_macro(c, prev):
            """stage1+2 of chunk c interleaved (via the filler queue) with
            the attention / oproj / LN / output work of chunk prev."""
            if c is not None:
                start_chunk(c)
                if prev is not None:
                    queue_attention(prev)
                for h in range(H):
                    emit_qk_head(c, h)
                for h in range(H):
                    emit_v_head(c, h)
                    if prev is not None:
                        if h < 8:
                            emit_oproj_nb(prev, 2 * h)
                            emit_oproj_nb(prev, 2 * h + 1)
                        elif h == 8:
                            emit_ln_stats(prev)
                            emit_out_nb(prev, 0)
                            emit_out_nb(prev, 1)
                        else:
                            emit_out_nb(prev, 2 * (h - 8))
                            emit_out_nb(prev, 2 * (h - 8) + 1)
                if prev is not None:
                    for nb in range(2 * (H - 8), NB):
                        emit_out_nb(prev, nb)
                    pop_filler(len(filler))
            else:
                # flush tail: overlap last chunk's attention with its output
                # projection by splitting oproj groups into sample halves
                queue_attention(prev)
                pop_filler(3 * (NBLK // 2) + 3)   # blocks 0..31 done
                for nb in range(NB):
                    emit_oproj_nb(prev, nb, half=0, pops=2)
                for nb in range(NB):
                    emit_oproj_nb(prev, nb, half=1, pops=2)
                pop_filler(len(filler))
                emit_ln_stats(prev)
                for nb in range(NB):
                    emit_out_nb(prev, nb, defer=False)
            if prev is not None:
                del st[prev]

        for c in range(nch):
            emit_macro(c, c - 1 if c > 0 else None)
        emit_macro(None, nch - 1)

    nc.compile()
    return nc


def _prep_host_inputs(q, k, v, Wq, bq, Wk, bk, Wv, bv, Wo, bo, gamma, beta,
                      ncores, bl):
    bf = ml_dtypes.bfloat16
    nch = bl // CHUNK

    def pack_xT(x):
        xT = np.ascontiguousarray(x.T).astype(bf)          # [D, B]
        view = xT.reshape(KO, 128, ncores, nch, CHUNK)
        return [np.ascontiguousarray(view[:, :, c].transpose(1, 2, 0, 3))
                for c in range(ncores)]

    def pack_w(W):
        return np.ascontiguousarray(
            W.reshape(KO, 128, H, 128).transpose(1, 2, 0, 3)).astype(bf)

    hh, dd = np.divmod(np.arange(D), HD)
    src = dd * H + hh
    Wo_p = Wo[src, :]
    wo_pack = np.ascontiguousarray(
        Wo_p.reshape(H, 128, NB, 128).transpose(1, 2, 0, 3)).astype(bf)

    r = np.arange(128)
    m = (r[:, None] % SBLK == r[None, :] % SBLK).astype(np.float32) / HD

    shared = {
        "wq": pack_w(Wq), "wk": pack_w(Wk), "wv": pack_w(Wv), "wo": wo_pack,
        "bqr": bq.reshape(1, D).astype(bf),
        "bkr": bk.reshape(1, D).astype(bf),
        "bvc": np.ascontiguousarray(
            bv.reshape(H, 128).T).astype(np.float32),
        "bor": bo.reshape(1, D).astype(bf),
        "gpk": np.ascontiguousarray(
            gamma.reshape(NB, 128).T).astype(np.float32),
        "bpk": np.ascontiguousarray(
            beta.reshape(NB, 128).T).astype(np.float32),
        "ident": np.eye(128, dtype=bf),
        "mask": m.astype(bf),
        "ones128": np.ones((128, 128), dtype=bf),
        "ones1": np.ones((1, CHUNK), dtype=bf),
    }
    return pack_xT(q), pack_xT(k), pack_xT(v), shared


def kernel(q, k, v, Wq, bq, Wk, bk, Wv, bv, Wo, bo, gamma, beta, _bl=None,
           _ncores=None, _trace=False, _tmpdir=None):
    ncores = _ncores or NCORES
    bl = _bl or (q.shape[0] // ncores)
    qTs, kTs, vTs, shared = _prep_host_inputs(
        q, k, v, Wq, bq, Wk, bk, Wv, bv, Wo, bo, gamma, beta, ncores, bl)
    nc = build_bass(bl)
    in_maps = []
    for c in range(ncores):
        m = dict(shared)
        m["qT"] = qTs[c]
        m["kT"] = kTs[c]
        m["vT"] = vTs[c]
        in_maps.append(m)
    res = run_bass_kernel_spmd(nc, in_maps, core_ids=list(range(ncores)),
                               trace=_trace, tmpdir=_tmpdir)
    outs = [r["out"] for r in res.results]
    full = np.concatenate(outs, axis=0)
    if _trace:
        kernel.last_results = res
    return full.astype(np.float32)
